# revision 19
# baseline (speedup 1.0000x reference)
"""ARMIN/TARDIS addressed-memory cell on 8 trn2 NeuronCores.

Data-parallel over batch: each core processes 32 of the 256 batch rows.
Weights are replicated. Score path (content addressing) runs as a 3-pass
bf16 hi/lo-split matmul (argmax margins are as small as 4e-4: bf16 flips
reads, fp32r at ~1e-4 is still too coarse, the 3-pass split gives ~4e-6);
the small q/r/score-dot matmuls run in exact fp32. The two big cell
matmuls run in bf16, and h_entry is gathered in fp32 via indirect DMA so
the r-output stays exact. The x/c part of matmul 1 plus its weight
streaming is interleaved into the score phase (it doesn't depend on the
addressed read), which keeps the DMA rings busy while the PE grinds on
the score matmuls.
"""

import numpy as np
import ml_dtypes
from contextlib import ExitStack

import concourse.bass as bass
import concourse.bacc as bacc
import concourse.tile as tile
from concourse import mybir
from concourse.bass_utils import run_bass_kernel_spmd
from concourse.masks import make_identity

F32 = mybir.dt.float32
BF16 = mybir.dt.bfloat16
U32 = mybir.dt.uint32
AF = mybir.ActivationFunctionType
ALU = mybir.AluOpType
AX = mybir.AxisListType

N_CORES = 8
B, X, H, M, KD = 256, 512, 1024, 128, 64
BC = B // N_CORES        # 32 batch rows per core
F = H // 4               # 256
BM = BC * M              # 4096
EPS = 1e-5
F_BIAS = 1.0
CHUNK = 512              # bm columns per score-path tile
NCHUNK = BM // CHUNK     # 8
NB = CHUNK // M          # batch rows per chunk (4)
KCAT = (X + 2 * H) // 128  # 20 contraction tiles for the cell matmuls


def _bcast_rows(handle_ap, lo, hi, rows=BC):
    """AP that reads dram vector[lo:hi] broadcast across `rows` partitions."""
    src = handle_ap[lo:hi]
    return bass.AP(tensor=src.tensor, offset=src.offset,
                   ap=[[0, rows]] + [list(d) for d in src.ap])


def build_nc(plain_affine):
    """plain_affine: skip LN gamma/beta and bias adds (they are 1/0)."""
    nc = bacc.Bacc("TRN2", target_bir_lowering=False, debug=False)
    P = {}

    def dp(name, shape, dtype=F32, out=False):
        P[name] = nc.declare_dram_parameter(name, list(shape), dtype, isOutput=out)
        return P[name]

    dp("hmemT_hi", [H, BM], BF16)   # [h, b*128+m] bf16 high part
    dp("hmemT_lo", [H, BM], BF16)   # residual (hmemT - hi) in bf16
    dp("hmem_flat", [BM, H])        # natural rows for the gather
    dp("xT", [X, BC]); dp("cT", [H, BC])
    dp("xT_bf", [X, BC], BF16); dp("cT_bf", [H, BC], BF16)
    dp("c_nat", [BC, H])
    dp("u_t", [BC, M]); dp("prev", [BC, M]); dp("gumbel_u", [BC, M])
    dp("keysT_pad", [128, M])
    dp("fc_w", [X + 2 * H + KD + M, F])
    dp("fc_b", [F]); dp("vec_a", [F])
    dp("veca_hi", [F], BF16); dp("veca_lo", [F], BF16)
    dp("fchm_hi", [H, F], BF16)     # fc_w rows 1600:2624 hi/lo split
    dp("fchm_lo", [H, F], BF16)
    dp("row_base", [BC, 1], U32)
    dp("w1bf", [X + 2 * H, 2 * H], BF16)
    dp("wfbf", [X + 2 * H, 5 * H], BF16)
    if not plain_affine:
        dp("bias1v", [2 * H]); dp("biasv", [5 * H])
        dp("ln1g", [5 * H]); dp("ln1b", [5 * H])
        dp("ln2g", [H]); dp("ln2b", [H])
        dp("ln3g", [2 * H]); dp("ln3b", [2 * H])
        dp("ln4g", [M]); dp("ln4b", [M])
    out_d = dp("out", [BC, 2 * H], out=True)
    score_d = nc.dram_tensor("score_bounce", [BM], F32)

    with ExitStack() as ctx:
        tc = ctx.enter_context(tile.TileContext(nc))
        consts = ctx.enter_context(tc.tile_pool(name="consts", bufs=1))
        hpool = ctx.enter_context(tc.tile_pool(name="hpool", bufs=2))
        wspool = ctx.enter_context(tc.tile_pool(name="wspool", bufs=6))
        hfpool = ctx.enter_context(tc.tile_pool(name="hfpool", bufs=2))
        bnpool = ctx.enter_context(tc.tile_pool(name="bnpool", bufs=4))
        zpool = ctx.enter_context(tc.tile_pool(name="zpool", bufs=2))
        bcpool = ctx.enter_context(tc.tile_pool(name="bcpool", bufs=3))
        pre_ps = ctx.enter_context(tc.tile_pool(name="pre_ps", bufs=3, space="PSUM"))
        tp_ps = ctx.enter_context(tc.tile_pool(name="tp_ps", bufs=2, space="PSUM"))
        cell_ps = ctx.enter_context(tc.tile_pool(name="cell_ps", bufs=1, space="PSUM"))

        def ln_rows(x_ap, d, out_ap, g_tile=None, b_tile=None):
            nsub = (d + 511) // 512
            sub = d // nsub
            stats = bnpool.tile([BC, nsub, 6], F32, tag="bn_stats", name="bn_stats")
            for i in range(nsub):
                nc.vector.bn_stats(out=stats[:, i, :],
                                   in_=x_ap[:, i * sub:(i + 1) * sub])
            mv = bnpool.tile([BC, 2], F32, tag="bn_mv", name="bn_mv")
            nc.vector.bn_aggr(out=mv[:], in_=stats[:])
            rstd = bnpool.tile([BC, 1], F32, tag="bn_rstd", name="bn_rstd")
            nc.scalar.activation(out=rstd[:], in_=mv[:, 1:2], func=AF.Sqrt,
                                 bias=eps_t[:], scale=1.0)
            nc.vector.reciprocal(out=rstd[:], in_=rstd[:])
            nc.vector.tensor_scalar(out=out_ap, in0=x_ap, scalar1=mv[:, 0:1],
                                    scalar2=rstd[:], op0=ALU.subtract, op1=ALU.mult)
            if g_tile is not None:
                nc.vector.tensor_mul(out=out_ap, in0=out_ap, in1=g_tile)
                nc.vector.tensor_add(out=out_ap, in0=out_ap, in1=b_tile)

        def bc_tile(pname, lo, hi):
            t = bcpool.tile([BC, hi - lo], F32, tag="bc", name="bc")
            nc.gpsimd.dma_start(out=t[:], in_=_bcast_rows(P[pname].ap(), lo, hi))
            return t

        # ---------- resident constants ----------
        ident = consts.tile([128, 128], F32, tag="ident")
        make_identity(nc, ident[:])
        eps_t = consts.tile([BC, 1], F32, tag="eps")
        nc.vector.memset(eps_t[:], EPS)
        e20_t = consts.tile([BC, 1], F32, tag="e20")
        nc.vector.memset(e20_t[:], 1e-20)

        fchm_hi = consts.tile([128, 8, F], BF16, tag="fchm_hi")
        nc.sync.dma_start(out=fchm_hi[:], in_=P["fchm_hi"].ap()[:, :]
                          .rearrange("(j p) n -> p j n", p=128))
        fchm_lo = consts.tile([128, 8, F], BF16, tag="fchm_lo")
        nc.sync.dma_start(out=fchm_lo[:], in_=P["fchm_lo"].ap()[:, :]
                          .rearrange("(j p) n -> p j n", p=128))
        fcxc = consts.tile([128, 12, F], F32, tag="fcxc")
        nc.sync.dma_start(out=fcxc[:], in_=P["fc_w"].ap()[0:1536, :]
                          .rearrange("(j p) n -> p j n", p=128))
        fckp = consts.tile([128, F], F32, tag="fckp")
        nc.sync.dma_start(out=fckp[:], in_=P["fc_w"].ap()[1536:1664, :])
        fcu = consts.tile([128, F], F32, tag="fcu")
        nc.sync.dma_start(out=fcu[:], in_=P["fc_w"].ap()[2624:2752, :])
        fcb = consts.tile([128, 2], F32, tag="fcb")
        nc.sync.dma_start(out=fcb[:], in_=P["fc_b"].ap().rearrange("(f p) -> p f", p=128))
        veca_hi = consts.tile([128, 2], BF16, tag="veca_hi")
        nc.sync.dma_start(out=veca_hi[:], in_=P["veca_hi"].ap().rearrange("(f p) -> p f", p=128))
        veca_lo = consts.tile([128, 2], BF16, tag="veca_lo")
        nc.sync.dma_start(out=veca_lo[:], in_=P["veca_lo"].ap().rearrange("(f p) -> p f", p=128))
        keysT = consts.tile([128, M], F32, tag="keysT")
        nc.sync.dma_start(out=keysT[:], in_=P["keysT_pad"].ap()[:])

        xT_f = consts.tile([128, 4, BC], F32, tag="xT_f")
        nc.sync.dma_start(out=xT_f[:], in_=P["xT"].ap()[:, :]
                          .rearrange("(j p) n -> p j n", p=128))
        cT_f = consts.tile([128, 8, BC], F32, tag="cT_f")
        nc.sync.dma_start(out=cT_f[:], in_=P["cT"].ap()[:, :]
                          .rearrange("(j p) n -> p j n", p=128))
        ck_bf = consts.tile([128, KCAT, BC], BF16, tag="ck_bf")    # ungated
        ckg_bf = consts.tile([128, KCAT, BC], BF16, tag="ckg_bf")  # gated
        c_nat = consts.tile([BC, H], F32, tag="c_nat")
        u_sb = consts.tile([BC, M], F32, tag="u_sb")
        nc.sync.dma_start(out=u_sb[:], in_=P["u_t"].ap()[:])
        prev_sb = consts.tile([BC, M], F32, tag="prev_sb")
        gum_sb = consts.tile([BC, M], F32, tag="gum_sb")
        rowb = consts.tile([BC, 1], U32, tag="rowb")

        # ---------- u_norm and its transpose ----------
        usq = consts.tile([BC, M], F32, tag="usq")
        nc.scalar.activation(out=usq[:], in_=u_sb[:], func=AF.Square)
        nrm = consts.tile([BC, 1], F32, tag="nrm")
        nc.vector.reduce_sum(out=nrm[:], in_=usq[:], axis=AX.X)
        nc.scalar.activation(out=nrm[:], in_=nrm[:], func=AF.Sqrt)
        nc.vector.tensor_scalar_max(nrm[:], nrm[:], 1e-12)
        nc.vector.reciprocal(out=nrm[:], in_=nrm[:])
        unorm = consts.tile([BC, M], F32, tag="unorm")
        nc.vector.tensor_scalar_mul(unorm[:], u_sb[:], nrm[:])
        tp = tp_ps.tile([128, BC], F32, tag="tp")
        nc.tensor.transpose(tp[:], unorm[:], ident[:BC, :BC])
        unT = consts.tile([128, BC], F32, tag="unT")
        nc.vector.tensor_copy(out=unT[:], in_=tp[:])

        # ---------- q = xc @ W_xc + u_norm @ W_u  (natural [b, f]) ----------
        qps = tp_ps.tile([BC, F], F32, tag="tp", name="qps")
        for k in range(4):
            nc.tensor.matmul(qps[:], lhsT=xT_f[:, k, :], rhs=fcxc[:, k, :],
                             start=(k == 0), stop=False)
        for k in range(8):
            nc.tensor.matmul(qps[:], lhsT=cT_f[:, k, :], rhs=fcxc[:, 4 + k, :],
                             start=False, stop=False)
        nc.tensor.matmul(qps[:], lhsT=unT[:], rhs=fcu[:], start=False, stop=True)
        q_nat = consts.tile([BC, F], F32, tag="q_nat")
        nc.vector.tensor_copy(out=q_nat[:], in_=qps[:])
        qT = consts.tile([128, 2, BC], F32, tag="qT")
        for f in range(2):
            tpq = tp_ps.tile([128, BC], F32, tag="tp", name="tpq")
            nc.tensor.transpose(tpq[:], q_nat[:, f * 128:(f + 1) * 128],
                                ident[:BC, :BC])
            nc.vector.tensor_copy(out=qT[:, f, :], in_=tpq[:])

        # ---------- r_km^T [f, m] = fc_kpad.T @ keysT_pad ----------
        rkT = consts.tile([128, 2, M], F32, tag="rkT")
        for f in range(2):
            rps = tp_ps.tile([128, M], F32, tag="tp", name="rps")
            nc.tensor.matmul(rps[:], lhsT=fckp[:, f * 128:(f + 1) * 128],
                             rhs=keysT[:], start=True, stop=True)
            nc.vector.tensor_copy(out=rkT[:, f, :], in_=rps[:])

        # ---------- score phase: pairs of chunks share each stationary ----------
        for cp in range(NCHUNK // 2):
            hts = []
            for half in range(2):
                ci = cp * 2 + half
                ht_hi = hpool.tile([128, 8, CHUNK], BF16, tag=f"ht_hi{half}",
                                   name="ht_hi")
                nc.sync.dma_start(out=ht_hi[:],
                                  in_=P["hmemT_hi"].ap()[:, ci * CHUNK:(ci + 1) * CHUNK]
                                  .rearrange("(j p) n -> p j n", p=128))
                ht_lo = hpool.tile([128, 8, CHUNK], BF16, tag=f"ht_lo{half}",
                                   name="ht_lo")
                nc.sync.dma_start(out=ht_lo[:],
                                  in_=P["hmemT_lo"].ap()[:, ci * CHUNK:(ci + 1) * CHUNK]
                                  .rearrange("(j p) n -> p j n", p=128))
                hts.append((ht_hi, ht_lo))
            sps2 = [tp_ps.tile([1, CHUNK], F32, tag="tp", name="sps") for _ in range(2)]
            for f in range(2):
                fs = slice(f * 128, (f + 1) * 128)
                ps2c = [pre_ps.tile([128, CHUNK], F32, tag="pre", name="pre")
                        for _ in range(2)]
                for term in range(3):
                    wsl = fchm_hi if term in (0, 2) else fchm_lo
                    for kh in range(8):
                        for half in range(2):
                            rhs = hts[half][1] if term == 2 else hts[half][0]
                            nc.tensor.matmul(ps2c[half][:],
                                             lhsT=wsl[:, kh, fs],
                                             rhs=rhs[:, kh, :],
                                             start=(term == 0 and kh == 0),
                                             stop=(term == 2 and kh == 7))
                for half in range(2):
                    ci = cp * 2 + half
                    ps = ps2c[half]
                    hf = hfpool.tile([128, CHUNK], F32, tag="hf", name="hf")
                    qb = qT[:, f, ci * NB:(ci + 1) * NB, None].to_broadcast(
                        [128, NB, M])
                    nc.vector.tensor_tensor(
                        out=hf[:].rearrange("p (b m) -> p b m", b=NB),
                        in0=ps[:].rearrange("p (b m) -> p b m", b=NB),
                        in1=qb, op=ALU.add)
                    rb = rkT[:, f, None, :].to_broadcast([128, NB, M])
                    nc.vector.tensor_tensor(
                        out=hf[:].rearrange("p (b m) -> p b m", b=NB),
                        in0=hf[:].rearrange("p (b m) -> p b m", b=NB),
                        in1=rb, op=ALU.add)
                    nc.scalar.activation(out=hf[:], in_=hf[:], func=AF.Tanh,
                                         bias=fcb[:, f:f + 1], scale=1.0)
                    hfh = hfpool.tile([128, CHUNK], BF16, tag="hfh", name="hfh")
                    nc.vector.tensor_copy(out=hfh[:], in_=hf[:])
                    hfl = hfpool.tile([128, CHUNK], BF16, tag="hfl", name="hfl")
                    nc.vector.tensor_sub(out=hfl[:], in0=hf[:], in1=hfh[:])
                    nc.tensor.matmul(sps2[half][:], lhsT=veca_hi[:, f:f + 1],
                                     rhs=hfh[:], start=(f == 0), stop=False)
                    nc.tensor.matmul(sps2[half][:], lhsT=veca_lo[:, f:f + 1],
                                     rhs=hfh[:], start=False, stop=False)
                    nc.tensor.matmul(sps2[half][:], lhsT=veca_hi[:, f:f + 1],
                                     rhs=hfl[:], start=False, stop=(f == 1))
            for half in range(2):
                ci = cp * 2 + half
                scs = hfpool.tile([1, CHUNK], F32, tag="scs", name="scs")
                nc.vector.tensor_copy(out=scs[:], in_=sps2[half][:])
                nc.sync.dma_start(
                    out=score_d.ap()[ci * CHUNK:(ci + 1) * CHUNK]
                    .rearrange("(a n) -> a n", a=1),
                    in_=scs[:])

        # deferred cell-phase const loads (keep the score-phase DMA queue lean)
        nc.sync.dma_start(out=ck_bf[:, 0:4, :], in_=P["xT_bf"].ap()[:, :]
                          .rearrange("(j p) n -> p j n", p=128))
        nc.sync.dma_start(out=ckg_bf[:, 0:4, :], in_=P["xT_bf"].ap()[:, :]
                          .rearrange("(j p) n -> p j n", p=128))
        nc.sync.dma_start(out=ck_bf[:, 4:12, :], in_=P["cT_bf"].ap()[:, :]
                          .rearrange("(j p) n -> p j n", p=128))
        nc.sync.dma_start(out=c_nat[:], in_=P["c_nat"].ap()[:])
        nc.sync.dma_start(out=prev_sb[:], in_=P["prev"].ap()[:])
        nc.sync.dma_start(out=gum_sb[:], in_=P["gumbel_u"].ap()[:])
        nc.sync.dma_start(out=rowb[:], in_=P["row_base"].ap()[:])

        # early-emitted weight stream: W1 then Wf prefetch through one pool
        w1_tiles = []
        for kq in range(5):
            for pz in range(2):
                w1t = wspool.tile([128, 4, 1024], BF16, tag="ws", name="w1t")
                nc.sync.dma_start(
                    out=w1t[:],
                    in_=P["w1bf"].ap()[kq * 512:(kq + 1) * 512,
                                       pz * 1024:(pz + 1) * 1024]
                    .rearrange("(j p) n -> p j n", p=128))
                w1_tiles.append(w1t)
        wf_tiles = []
        for pz in range(5):
            for kq in range(5):
                wft = wspool.tile([128, 4, 1024], BF16, tag="ws", name="wft")
                nc.sync.dma_start(
                    out=wft[:],
                    in_=P["wfbf"].ap()[kq * 512:(kq + 1) * 512,
                                       pz * 1024:(pz + 1) * 1024]
                    .rearrange("(j p) n -> p j n", p=128))
                wf_tiles.append(wft)

        score_bm = consts.tile([BC, M], F32, tag="score_bm")
        nc.sync.dma_start(out=score_bm[:],
                          in_=score_d.ap().rearrange("(b m) -> b m", b=BC))

        # score -= prev*100 ; ln4 ; + gumbel ; argmax
        p100 = consts.tile([BC, M], F32, tag="p100")
        nc.vector.tensor_scalar_mul(p100[:], prev_sb[:], 100.0)
        nc.vector.tensor_sub(out=score_bm[:], in0=score_bm[:], in1=p100[:])
        if plain_affine:
            ln_rows(score_bm[:], M, score_bm[:])
        else:
            g4 = bc_tile("ln4g", 0, M)
            b4 = bc_tile("ln4b", 0, M)
            ln_rows(score_bm[:], M, score_bm[:], g4[:], b4[:])
        gt = consts.tile([BC, M], F32, tag="gt")
        nc.scalar.activation(out=gt[:], in_=gum_sb[:], func=AF.Ln, bias=e20_t[:])
        nc.vector.tensor_scalar(out=gt[:], in0=gt[:], scalar1=-1.0, scalar2=1e-20,
                                op0=ALU.mult, op1=ALU.add)
        nc.scalar.activation(out=gt[:], in_=gt[:], func=AF.Ln)
        nc.vector.tensor_sub(out=score_bm[:], in0=score_bm[:], in1=gt[:])
        mx8 = consts.tile([BC, 8], F32, tag="mx8")
        nc.vector.max(out=mx8[:], in_=score_bm[:])
        mi8 = consts.tile([BC, 8], U32, tag="mi8")
        nc.vector.max_index(out=mi8[:], in_max=mx8[:], in_values=score_bm[:])
        flat = consts.tile([BC, 1], U32, tag="flat")
        nc.vector.tensor_tensor(out=flat[:], in0=rowb[:], in1=mi8[:, 0:1], op=ALU.add)

        # gather h_entry rows (fp32 exact)
        h_ent = consts.tile([BC, H], F32, tag="h_ent")
        nc.gpsimd.indirect_dma_start(
            out=h_ent[:], out_offset=None, in_=P["hmem_flat"].ap(),
            in_offset=bass.IndirectOffsetOnAxis(ap=flat[:, :1], axis=0))

        # h_entry^T tiles (fp32 for gating, bf16 for matmul 1)
        hT_f = consts.tile([128, 8, BC], F32, tag="hT_f")
        for kh in range(8):
            tph = tp_ps.tile([128, BC], F32, tag="tp", name="tph")
            nc.tensor.transpose(tph[:], h_ent[:, kh * 128:(kh + 1) * 128],
                                ident[:BC, :BC])
            nc.vector.tensor_copy(out=hT_f[:, kh, :], in_=tph[:])
            nc.vector.tensor_copy(out=ck_bf[:, 12 + kh, :], in_=tph[:])

        # ---------- matmul 1: all k-tiles (weights were prefetched) ----------
        g1 = consts.tile([BC, 2 * H], F32, tag="g1")
        for pz in range(2):
            ps1 = cell_ps.tile([BC, 1024], F32, tag="cellps", name="ps1")
            for kq in range(5):
                w1t = w1_tiles[kq * 2 + pz]
                for j in range(4):
                    k = kq * 4 + j
                    for nn in range(2):
                        nc.tensor.matmul(ps1[:, nn * 512:(nn + 1) * 512],
                                         lhsT=ck_bf[:, k, :],
                                         rhs=w1t[:, j, nn * 512:(nn + 1) * 512],
                                         start=(k == 0), stop=(k == KCAT - 1))
            z1p = zpool.tile([BC, 1024], F32, tag="z1p", name="z1p")
            if plain_affine:
                nc.vector.tensor_copy(out=z1p[:], in_=ps1[:])
                ln_rows(z1p[:], 1024, z1p[:])
            else:
                b1c = bc_tile("bias1v", pz * 1024, (pz + 1) * 1024)
                nc.vector.tensor_add(out=z1p[:], in0=ps1[:], in1=b1c[:])
                g3 = bc_tile("ln3g", pz * 1024, (pz + 1) * 1024)
                b3 = bc_tile("ln3b", pz * 1024, (pz + 1) * 1024)
                ln_rows(z1p[:], 1024, z1p[:], g3[:], b3[:])
            nc.scalar.activation(out=g1[:, pz * 1024:(pz + 1) * 1024], in_=z1p[:],
                                 func=AF.Sigmoid)

        # gate: ckg[4+t] = (cT | h_entry^T)[t] * g1^T[t]   (bf16 cast on write)
        for t in range(16):
            tpg = tp_ps.tile([128, BC], F32, tag="tp", name="tpg")
            nc.tensor.transpose(tpg[:], g1[:, t * 128:(t + 1) * 128], ident[:BC, :BC])
            src = cT_f[:, t, :] if t < 8 else hT_f[:, t - 8, :]
            nc.vector.tensor_mul(out=ckg_bf[:, 4 + t, :], in0=src, in1=tpg[:])

        # ---------- matmul 2: z = gated @ W_full, ln1 per chunk ----------
        zln = [consts.tile([BC, 1024], F32, tag=f"zln{i}", name=f"zln{i}")
               for i in range(5)]
        for pz in range(5):
            ps2 = cell_ps.tile([BC, 1024], F32, tag="cellps", name="ps2")
            for kq in range(5):
                wft = wf_tiles[pz * 5 + kq]
                for j in range(4):
                    k = kq * 4 + j
                    for nn in range(2):
                        nc.tensor.matmul(ps2[:, nn * 512:(nn + 1) * 512],
                                         lhsT=ckg_bf[:, k, :],
                                         rhs=wft[:, j, nn * 512:(nn + 1) * 512],
                                         start=(k == 0), stop=(k == KCAT - 1))
            if plain_affine:
                nc.vector.tensor_copy(out=zln[pz][:], in_=ps2[:])
                ln_rows(zln[pz][:], 1024, zln[pz][:])
            else:
                bvc = bc_tile("biasv", pz * 1024, (pz + 1) * 1024)
                nc.vector.tensor_add(out=zln[pz][:], in0=ps2[:], in1=bvc[:])
                g1c = bc_tile("ln1g", pz * 1024, (pz + 1) * 1024)
                b1cc = bc_tile("ln1b", pz * 1024, (pz + 1) * 1024)
                ln_rows(zln[pz][:], 1024, zln[pz][:], g1c[:], b1cc[:])

        # ---------- cell math ----------
        zi, zj, zf, zo, zom = zln
        nc.scalar.activation(out=zf[:], in_=zf[:], func=AF.Sigmoid, bias=F_BIAS)
        nc.scalar.activation(out=zi[:], in_=zi[:], func=AF.Sigmoid)
        nc.scalar.activation(out=zj[:], in_=zj[:], func=AF.Tanh)
        nc.vector.tensor_mul(out=zf[:], in0=c_nat[:], in1=zf[:])
        nc.vector.tensor_mul(out=zi[:], in0=zi[:], in1=zj[:])
        nc.vector.tensor_add(out=zf[:], in0=zf[:], in1=zi[:])
        if plain_affine:
            ln_rows(zf[:], H, zf[:])
        else:
            g2c = bc_tile("ln2g", 0, H)
            b2c = bc_tile("ln2b", 0, H)
            ln_rows(zf[:], H, zf[:], g2c[:], b2c[:])
        nc.scalar.activation(out=zj[:], in_=zf[:], func=AF.Tanh)
        nc.scalar.activation(out=zo[:], in_=zo[:], func=AF.Sigmoid)
        nc.vector.tensor_mul(out=zj[:], in0=zj[:], in1=zo[:])
        rh = consts.tile([BC, H], F32, tag="rh")
        nc.scalar.activation(out=rh[:], in_=h_ent[:], func=AF.Tanh)
        nc.scalar.activation(out=zom[:], in_=zom[:], func=AF.Sigmoid)
        nc.vector.tensor_mul(out=rh[:], in0=rh[:], in1=zom[:])

        nc.sync.dma_start(out=out_d.ap()[:, 0:H], in_=zj[:])
        nc.sync.dma_start(out=out_d.ap()[:, H:2 * H], in_=rh[:])

    nc.compile()
    return nc


def build_nc_colsplit():
    """Column-split cell across the 8 cores (plain-affine only).

    Each core computes the full 256-row batch for its own 1/8 of the
    W_full1/W_full columns (within-chunk 128-col blocks, so LN statistics
    AllReduce buffers are identical in shape on every core and the SPMD
    program never depends on the core id). h_entry and the gate are
    AllGathered; the per-core h_entry column slice for the r-output is
    extracted with a host-provided one-hot selection matmul.
    """
    GRP = [list(range(N_CORES))]
    nc = bacc.Bacc("TRN2", target_bir_lowering=False, debug=False,
                   num_devices=N_CORES)
    P = {}

    def dp(name, shape, dtype=F32, out=False):
        P[name] = nc.declare_dram_parameter(name, list(shape), dtype, isOutput=out)
        return P[name]

    dp("hmemT_hi", [H, BM], BF16)
    dp("hmemT_lo", [H, BM], BF16)
    dp("hmem_flat", [BM, H])
    dp("xT", [X, BC]); dp("cT", [H, BC])
    dp("u_t", [BC, M]); dp("prev", [BC, M]); dp("gumbel_u", [BC, M])
    dp("keysT_pad", [128, M])
    dp("fc_w", [X + 2 * H + KD + M, F])
    dp("fc_b", [F])
    dp("veca_hi", [F], BF16); dp("veca_lo", [F], BF16)
    dp("fchm_hi", [H, F], BF16)
    dp("fchm_lo", [H, F], BF16)
    dp("row_base", [BC, 1], U32)
    dp("xT_bf_full", [X, B], BF16)
    dp("cT_bf_full", [H, B], BF16)
    dp("c_cols", [B, 128])
    dp("w1_cols", [X + 2 * H, 256], BF16)
    dp("wf_cols", [X + 2 * H, 640], BF16)
    dp("s_sel", [H, 128])
    out_d = dp("out", [B, 256], out=True)
    score_d = nc.dram_tensor("score_bounce", [BM], F32)
    ag_he_in = nc.dram_tensor("ag_he_in", [BC, H], F32)
    ag_he_out = nc.dram_tensor("ag_he_out", [B, H], F32, addr_space="Shared")
    ag_g1_in = nc.dram_tensor("ag_g1_in", [256, B], F32)
    ag_g1_out = nc.dram_tensor("ag_g1_out", [2048, B], F32, addr_space="Shared")
    ar3_in = nc.dram_tensor("ar3_in", [B, 4], F32)
    ar3_out = nc.dram_tensor("ar3_out", [B, 4], F32, addr_space="Shared")
    ar1_in = nc.dram_tensor("ar1_in", [B, 10], F32)
    ar1_out = nc.dram_tensor("ar1_out", [B, 10], F32, addr_space="Shared")
    ar2_in = nc.dram_tensor("ar2_in", [B, 2], F32)
    ar2_out = nc.dram_tensor("ar2_out", [B, 2], F32, addr_space="Shared")

    with ExitStack() as ctx:
        tc = ctx.enter_context(tile.TileContext(nc))
        consts = ctx.enter_context(tc.tile_pool(name="consts", bufs=1))
        hpool = ctx.enter_context(tc.tile_pool(name="hpool", bufs=2))
        hfpool = ctx.enter_context(tc.tile_pool(name="hfpool", bufs=2))
        bnpool = ctx.enter_context(tc.tile_pool(name="bnpool", bufs=4))
        sqpool = ctx.enter_context(tc.tile_pool(name="sqpool", bufs=1))
        pre_ps = ctx.enter_context(tc.tile_pool(name="pre_ps", bufs=2, space="PSUM"))
        tp_ps = ctx.enter_context(tc.tile_pool(name="tp_ps", bufs=2, space="PSUM"))
        cell_ps = ctx.enter_context(tc.tile_pool(name="cell_ps", bufs=2, space="PSUM"))

        def ln_rows(x_ap, d, out_ap):
            nsub = (d + 511) // 512
            sub = d // nsub
            stats = bnpool.tile([BC, nsub, 6], F32, tag="bn_stats", name="bn_stats")
            for i in range(nsub):
                nc.vector.bn_stats(out=stats[:, i, :],
                                   in_=x_ap[:, i * sub:(i + 1) * sub])
            mv = bnpool.tile([BC, 2], F32, tag="bn_mv", name="bn_mv")
            nc.vector.bn_aggr(out=mv[:], in_=stats[:])
            rstd = bnpool.tile([BC, 1], F32, tag="bn_rstd", name="bn_rstd")
            nc.scalar.activation(out=rstd[:], in_=mv[:, 1:2], func=AF.Sqrt,
                                 bias=eps_t[:], scale=1.0)
            nc.vector.reciprocal(out=rstd[:], in_=rstd[:])
            nc.vector.tensor_scalar(out=out_ap, in0=x_ap, scalar1=mv[:, 0:1],
                                    scalar2=rstd[:], op0=ALU.subtract, op1=ALU.mult)

        # ---------- resident constants (score path) ----------
        ident = consts.tile([128, 128], F32, tag="ident")
        make_identity(nc, ident[:])
        eps_t = consts.tile([BC, 1], F32, tag="eps")
        nc.vector.memset(eps_t[:], EPS)
        eps128 = consts.tile([128, 1], F32, tag="eps128")
        nc.vector.memset(eps128[:], EPS)
        e20_t = consts.tile([BC, 1], F32, tag="e20")
        nc.vector.memset(e20_t[:], 1e-20)

        fchm_hi = consts.tile([128, 8, F], BF16, tag="fchm_hi")
        nc.sync.dma_start(out=fchm_hi[:], in_=P["fchm_hi"].ap()[:, :]
                          .rearrange("(j p) n -> p j n", p=128))
        fchm_lo = consts.tile([128, 8, F], BF16, tag="fchm_lo")
        nc.sync.dma_start(out=fchm_lo[:], in_=P["fchm_lo"].ap()[:, :]
                          .rearrange("(j p) n -> p j n", p=128))
        fcxc = consts.tile([128, 12, F], F32, tag="fcxc")
        nc.sync.dma_start(out=fcxc[:], in_=P["fc_w"].ap()[0:1536, :]
                          .rearrange("(j p) n -> p j n", p=128))
        fckp = consts.tile([128, F], F32, tag="fckp")
        nc.sync.dma_start(out=fckp[:], in_=P["fc_w"].ap()[1536:1664, :])
        fcu = consts.tile([128, F], F32, tag="fcu")
        nc.sync.dma_start(out=fcu[:], in_=P["fc_w"].ap()[2624:2752, :])
        fcb = consts.tile([128, 2], F32, tag="fcb")
        nc.sync.dma_start(out=fcb[:], in_=P["fc_b"].ap().rearrange("(f p) -> p f", p=128))
        veca_hi = consts.tile([128, 2], BF16, tag="veca_hi")
        nc.sync.dma_start(out=veca_hi[:], in_=P["veca_hi"].ap().rearrange("(f p) -> p f", p=128))
        veca_lo = consts.tile([128, 2], BF16, tag="veca_lo")
        nc.sync.dma_start(out=veca_lo[:], in_=P["veca_lo"].ap().rearrange("(f p) -> p f", p=128))
        keysT = consts.tile([128, M], F32, tag="keysT")
        nc.sync.dma_start(out=keysT[:], in_=P["keysT_pad"].ap()[:])
        xT_f = consts.tile([128, 4, BC], F32, tag="xT_f")
        nc.sync.dma_start(out=xT_f[:], in_=P["xT"].ap()[:, :]
                          .rearrange("(j p) n -> p j n", p=128))
        cT_f = consts.tile([128, 8, BC], F32, tag="cT_f")
        nc.sync.dma_start(out=cT_f[:], in_=P["cT"].ap()[:, :]
                          .rearrange("(j p) n -> p j n", p=128))
        u_sb = consts.tile([BC, M], F32, tag="u_sb")
        nc.sync.dma_start(out=u_sb[:], in_=P["u_t"].ap()[:])
        prev_sb = consts.tile([BC, M], F32, tag="prev_sb")
        gum_sb = consts.tile([BC, M], F32, tag="gum_sb")
        rowb = consts.tile([BC, 1], U32, tag="rowb")

        # ---------- u_norm / q / r_km (unchanged score preamble) ----------
        usq = consts.tile([BC, M], F32, tag="usq")
        nc.scalar.activation(out=usq[:], in_=u_sb[:], func=AF.Square)
        nrm = consts.tile([BC, 1], F32, tag="nrm")
        nc.vector.reduce_sum(out=nrm[:], in_=usq[:], axis=AX.X)
        nc.scalar.activation(out=nrm[:], in_=nrm[:], func=AF.Sqrt)
        nc.vector.tensor_scalar_max(nrm[:], nrm[:], 1e-12)
        nc.vector.reciprocal(out=nrm[:], in_=nrm[:])
        unorm = consts.tile([BC, M], F32, tag="unorm")
        nc.vector.tensor_scalar_mul(unorm[:], u_sb[:], nrm[:])
        tp = tp_ps.tile([128, BC], F32, tag="tp")
        nc.tensor.transpose(tp[:], unorm[:], ident[:BC, :BC])
        unT = consts.tile([128, BC], F32, tag="unT")
        nc.vector.tensor_copy(out=unT[:], in_=tp[:])

        qps = tp_ps.tile([BC, F], F32, tag="tp", name="qps")
        for k in range(4):
            nc.tensor.matmul(qps[:], lhsT=xT_f[:, k, :], rhs=fcxc[:, k, :],
                             start=(k == 0), stop=False)
        for k in range(8):
            nc.tensor.matmul(qps[:], lhsT=cT_f[:, k, :], rhs=fcxc[:, 4 + k, :],
                             start=False, stop=False)
        nc.tensor.matmul(qps[:], lhsT=unT[:], rhs=fcu[:], start=False, stop=True)
        q_nat = consts.tile([BC, F], F32, tag="q_nat")
        nc.vector.tensor_copy(out=q_nat[:], in_=qps[:])
        qT = consts.tile([128, 2, BC], F32, tag="qT")
        for f in range(2):
            tpq = tp_ps.tile([128, BC], F32, tag="tp", name="tpq")
            nc.tensor.transpose(tpq[:], q_nat[:, f * 128:(f + 1) * 128],
                                ident[:BC, :BC])
            nc.vector.tensor_copy(out=qT[:, f, :], in_=tpq[:])
        rkT = consts.tile([128, 2, M], F32, tag="rkT")
        for f in range(2):
            rps = tp_ps.tile([128, M], F32, tag="tp", name="rps")
            nc.tensor.matmul(rps[:], lhsT=fckp[:, f * 128:(f + 1) * 128],
                             rhs=keysT[:], start=True, stop=True)
            nc.vector.tensor_copy(out=rkT[:, f, :], in_=rps[:])

        # ---------- score phase (same as data-parallel build) ----------
        for cp in range(NCHUNK // 2):
            hts = []
            for half in range(2):
                ci = cp * 2 + half
                ht_hi = hpool.tile([128, 8, CHUNK], BF16, tag=f"ht_hi{half}",
                                   name="ht_hi")
                nc.sync.dma_start(out=ht_hi[:],
                                  in_=P["hmemT_hi"].ap()[:, ci * CHUNK:(ci + 1) * CHUNK]
                                  .rearrange("(j p) n -> p j n", p=128))
                ht_lo = hpool.tile([128, 8, CHUNK], BF16, tag=f"ht_lo{half}",
                                   name="ht_lo")
                nc.sync.dma_start(out=ht_lo[:],
                                  in_=P["hmemT_lo"].ap()[:, ci * CHUNK:(ci + 1) * CHUNK]
                                  .rearrange("(j p) n -> p j n", p=128))
                hts.append((ht_hi, ht_lo))
            sps2 = [tp_ps.tile([1, CHUNK], F32, tag="tp", name="sps") for _ in range(2)]
            for f in range(2):
                fs = slice(f * 128, (f + 1) * 128)
                ps2c = [pre_ps.tile([128, CHUNK], F32, tag="pre", name="pre")
                        for _ in range(2)]
                for term in range(3):
                    wsl = fchm_hi if term in (0, 2) else fchm_lo
                    for kh in range(8):
                        for half in range(2):
                            rhs = hts[half][1] if term == 2 else hts[half][0]
                            nc.tensor.matmul(ps2c[half][:],
                                             lhsT=wsl[:, kh, fs],
                                             rhs=rhs[:, kh, :],
                                             start=(term == 0 and kh == 0),
                                             stop=(term == 2 and kh == 7))
                for half in range(2):
                    ci = cp * 2 + half
                    ps = ps2c[half]
                    hf = hfpool.tile([128, CHUNK], F32, tag="hf", name="hf")
                    qb = qT[:, f, ci * NB:(ci + 1) * NB, None].to_broadcast(
                        [128, NB, M])
                    nc.vector.tensor_tensor(
                        out=hf[:].rearrange("p (b m) -> p b m", b=NB),
                        in0=ps[:].rearrange("p (b m) -> p b m", b=NB),
                        in1=qb, op=ALU.add)
                    rb = rkT[:, f, None, :].to_broadcast([128, NB, M])
                    nc.vector.tensor_tensor(
                        out=hf[:].rearrange("p (b m) -> p b m", b=NB),
                        in0=hf[:].rearrange("p (b m) -> p b m", b=NB),
                        in1=rb, op=ALU.add)
                    nc.scalar.activation(out=hf[:], in_=hf[:], func=AF.Tanh,
                                         bias=fcb[:, f:f + 1], scale=1.0)
                    hfh = hfpool.tile([128, CHUNK], BF16, tag="hfh", name="hfh")
                    nc.vector.tensor_copy(out=hfh[:], in_=hf[:])
                    hfl = hfpool.tile([128, CHUNK], BF16, tag="hfl", name="hfl")
                    nc.vector.tensor_sub(out=hfl[:], in0=hf[:], in1=hfh[:])
                    nc.tensor.matmul(sps2[half][:], lhsT=veca_hi[:, f:f + 1],
                                     rhs=hfh[:], start=(f == 0), stop=False)
                    nc.tensor.matmul(sps2[half][:], lhsT=veca_lo[:, f:f + 1],
                                     rhs=hfh[:], start=False, stop=False)
                    nc.tensor.matmul(sps2[half][:], lhsT=veca_hi[:, f:f + 1],
                                     rhs=hfl[:], start=False, stop=(f == 1))
            for half in range(2):
                ci = cp * 2 + half
                scs = hfpool.tile([1, CHUNK], F32, tag="scs", name="scs")
                nc.vector.tensor_copy(out=scs[:], in_=sps2[half][:])
                nc.sync.dma_start(
                    out=score_d.ap()[ci * CHUNK:(ci + 1) * CHUNK]
                    .rearrange("(a n) -> a n", a=1),
                    in_=scs[:])

        # ---------- cell-phase constant loads (prefetch during score) ----------
        w1sb = consts.tile([128, 20, 256], BF16, tag="w1sb")
        nc.sync.dma_start(out=w1sb[:], in_=P["w1_cols"].ap()[:, :]
                          .rearrange("(j p) n -> p j n", p=128))
        ckF = consts.tile([128, KCAT, 2, 128], BF16, tag="ckF")
        ckgF = consts.tile([128, KCAT, 2, 128], BF16, tag="ckgF")
        nc.sync.dma_start(out=ckF[:, 0:4, :, :], in_=P["xT_bf_full"].ap()[:, :]
                          .rearrange("(j p) (o n) -> p j o n", p=128, n=128))
        nc.sync.dma_start(out=ckgF[:, 0:4, :, :], in_=P["xT_bf_full"].ap()[:, :]
                          .rearrange("(j p) (o n) -> p j o n", p=128, n=128))
        nc.sync.dma_start(out=ckF[:, 4:12, :, :], in_=P["cT_bf_full"].ap()[:, :]
                          .rearrange("(j p) (o n) -> p j o n", p=128, n=128))
        s_sb = consts.tile([128, 8, 128], F32, tag="s_sb")
        nc.sync.dma_start(out=s_sb[:], in_=P["s_sel"].ap()[:, :]
                          .rearrange("(j p) n -> p j n", p=128))
        ccol_sb = consts.tile([128, 2, 128], F32, tag="ccol_sb")
        nc.sync.dma_start(out=ccol_sb[:], in_=P["c_cols"].ap()[:, :]
                          .rearrange("(o p) n -> p o n", p=128))
        nc.sync.dma_start(out=prev_sb[:], in_=P["prev"].ap()[:])
        nc.sync.dma_start(out=gum_sb[:], in_=P["gumbel_u"].ap()[:])
        nc.sync.dma_start(out=rowb[:], in_=P["row_base"].ap()[:])

        # ---------- score tail: ln4, gumbel, argmax, gather ----------
        score_bm = consts.tile([BC, M], F32, tag="score_bm")
        nc.sync.dma_start(out=score_bm[:],
                          in_=score_d.ap().rearrange("(b m) -> b m", b=BC))
        p100 = consts.tile([BC, M], F32, tag="p100")
        nc.vector.tensor_scalar_mul(p100[:], prev_sb[:], 100.0)
        nc.vector.tensor_sub(out=score_bm[:], in0=score_bm[:], in1=p100[:])
        ln_rows(score_bm[:], M, score_bm[:])
        gt = consts.tile([BC, M], F32, tag="gt")
        nc.scalar.activation(out=gt[:], in_=gum_sb[:], func=AF.Ln, bias=e20_t[:])
        nc.vector.tensor_scalar(out=gt[:], in0=gt[:], scalar1=-1.0, scalar2=1e-20,
                                op0=ALU.mult, op1=ALU.add)
        nc.scalar.activation(out=gt[:], in_=gt[:], func=AF.Ln)
        nc.vector.tensor_sub(out=score_bm[:], in0=score_bm[:], in1=gt[:])
        mx8 = consts.tile([BC, 8], F32, tag="mx8")
        nc.vector.max(out=mx8[:], in_=score_bm[:])
        mi8 = consts.tile([BC, 8], U32, tag="mi8")
        nc.vector.max_index(out=mi8[:], in_max=mx8[:], in_values=score_bm[:])
        flat = consts.tile([BC, 1], U32, tag="flat")
        nc.vector.tensor_tensor(out=flat[:], in0=rowb[:], in1=mi8[:, 0:1], op=ALU.add)
        h_ent = consts.tile([BC, H], F32, tag="h_ent")
        nc.gpsimd.indirect_dma_start(
            out=h_ent[:], out_offset=None, in_=P["hmem_flat"].ap(),
            in_offset=bass.IndirectOffsetOnAxis(ap=flat[:, :1], axis=0))

        # ---------- AllGather h_entry, build full-batch h_entry^T ----------
        nc.sync.dma_start(out=ag_he_in.ap()[:], in_=h_ent[:])
        nc.gpsimd.collective_compute(
            "AllGather", ALU.bypass, replica_groups=GRP,
            ins=[ag_he_in.ap()[:]], outs=[ag_he_out.ap()[:]])
        he_nat = consts.tile([128, 2, H], F32, tag="he_nat")
        nc.sync.dma_start(out=he_nat[:], in_=ag_he_out.ap()
                          .rearrange("(o p) n -> p o n", p=128))
        heT_f = consts.tile([128, 8, 2, 128], F32, tag="heT_f")
        for bt in range(2):
            for kh in range(8):
                tph = tp_ps.tile([128, 128], F32, tag="tp", name="tph")
                nc.tensor.transpose(tph[:], he_nat[:, bt, kh * 128:(kh + 1) * 128],
                                    ident[:])
                nc.vector.tensor_copy(out=heT_f[:, kh, bt, :], in_=tph[:])
                nc.vector.tensor_copy(out=ckF[:, 12 + kh, bt, :], in_=tph[:])

        # ---------- matmul 1 (full batch, own 256 W1 columns) ----------
        z1c = consts.tile([128, 2, 256], F32, tag="z1c")
        for bt in range(2):
            ps1 = cell_ps.tile([128, 256], F32, tag="cellps2", name="ps1")
            for k in range(KCAT):
                nc.tensor.matmul(ps1[:], lhsT=ckF[:, k, bt, :], rhs=w1sb[:, k, :],
                                 start=(k == 0), stop=(k == KCAT - 1))
            nc.vector.tensor_copy(out=z1c[:, bt, :], in_=ps1[:])

        # ln3 stats: own 128 cols per chunk -> AllReduce partial sums
        z1sq = sqpool.tile([128, 2, 256], F32, tag="zsq1", name="z1sq")
        nc.vector.tensor_mul(out=z1sq[:], in0=z1c[:], in1=z1c[:])
        st3 = consts.tile([128, 2, 4], F32, tag="st3")
        nc.vector.reduce_sum(out=st3[:, :, 0:2],
                             in_=z1c[:].rearrange("p o (c n) -> p o c n", c=2),
                             axis=AX.X)
        nc.vector.reduce_sum(out=st3[:, :, 2:4],
                             in_=z1sq[:].rearrange("p o (c n) -> p o c n", c=2),
                             axis=AX.X)
        nc.sync.dma_start(out=ar3_in.ap().rearrange("(o p) s -> p o s", p=128),
                          in_=st3[:])
        nc.gpsimd.collective_compute(
            "AllReduce", ALU.add, replica_groups=GRP,
            ins=[ar3_in.ap()[:]], outs=[ar3_out.ap()[:]])
        st3r = consts.tile([128, 2, 4], F32, tag="st3r")
        nc.sync.dma_start(out=st3r[:], in_=ar3_out.ap()
                          .rearrange("(o p) s -> p o s", p=128))
        g1c = consts.tile([128, 2, 256], F32, tag="g1c")
        for bt in range(2):
            for ch in range(2):
                mean = bnpool.tile([128, 1], F32, tag="m128", name="mean")
                nc.scalar.activation(out=mean[:], in_=st3r[:, bt, ch:ch + 1],
                                     func=AF.Identity, scale=1.0 / 1024)
                msq = bnpool.tile([128, 1], F32, tag="msq128", name="msq")
                nc.vector.tensor_mul(out=msq[:], in0=mean[:], in1=mean[:])
                var = bnpool.tile([128, 1], F32, tag="v128", name="var")
                nc.scalar.activation(out=var[:], in_=st3r[:, bt, 2 + ch:3 + ch],
                                     func=AF.Identity, scale=1.0 / 1024)
                nc.vector.tensor_sub(out=var[:], in0=var[:], in1=msq[:])
                nc.scalar.activation(out=var[:], in_=var[:], func=AF.Sqrt,
                                     bias=eps128[:], scale=1.0)
                nc.vector.reciprocal(out=var[:], in_=var[:])
                sl = z1c[:, bt, ch * 128:(ch + 1) * 128]
                nc.vector.tensor_scalar(out=g1c[:, bt, ch * 128:(ch + 1) * 128],
                                        in0=sl, scalar1=mean[:], scalar2=var[:],
                                        op0=ALU.subtract, op1=ALU.mult)
        nc.scalar.activation(out=g1c[:], in_=g1c[:], func=AF.Sigmoid)

        # ---------- AllGather the gate (transposed) ----------
        g1to = consts.tile([128, 2, 2, 128], F32, tag="g1to")
        for lt in range(2):
            for bt in range(2):
                tpg = tp_ps.tile([128, 128], F32, tag="tp", name="tpg")
                nc.tensor.transpose(tpg[:], g1c[:, bt, lt * 128:(lt + 1) * 128],
                                    ident[:])
                nc.vector.tensor_copy(out=g1to[:, lt, bt, :], in_=tpg[:])
        nc.sync.dma_start(
            out=ag_g1_in.ap().rearrange("(lt p) (bt n) -> p lt bt n", p=128, n=128),
            in_=g1to[:])
        nc.gpsimd.collective_compute(
            "AllGather", ALU.bypass, replica_groups=GRP,
            ins=[ag_g1_in.ap()[:]], outs=[ag_g1_out.ap()[:]])
        g1T_sb = consts.tile([128, 16, 2, 128], BF16, tag="g1T_sb")
        nc.gpsimd.dma_start(out=g1T_sb[:], in_=ag_g1_out.ap()
                            .rearrange("(T p) (o n) -> p T o n", p=128, n=128))

        # gating: t<8 gates c (ckF[4+t]), t>=8 gates h_entry (heT)
        for t in range(16):
            T = 2 * t if t < 8 else 2 * (t - 8) + 1
            for bt in range(2):
                src = ckF[:, 4 + t, bt, :] if t < 8 else heT_f[:, t - 8, bt, :]
                nc.vector.tensor_mul(out=ckgF[:, 4 + t, bt, :], in0=src,
                                     in1=g1T_sb[:, T, bt, :])

        # ---------- matmul 2 (full batch, own 640 Wf columns) ----------
        z_sb = consts.tile([128, 2, 5, 128], F32, tag="z_sb")
        ps2 = [cell_ps.tile([128, 640], F32, tag="cellps2", name=f"ps2_{i}")
               for i in range(2)]
        for kq in range(5):
            wfq = hfpool.tile([128, 4, 640], BF16, tag="wfq", name="wfq")
            nc.sync.dma_start(out=wfq[:],
                              in_=P["wf_cols"].ap()[kq * 512:(kq + 1) * 512, :]
                              .rearrange("(j p) n -> p j n", p=128))
            for j in range(4):
                k = kq * 4 + j
                for bt in range(2):
                    nc.tensor.matmul(ps2[bt][:, 0:512], lhsT=ckgF[:, k, bt, :],
                                     rhs=wfq[:, j, 0:512],
                                     start=(k == 0), stop=(k == KCAT - 1))
                    nc.tensor.matmul(ps2[bt][:, 512:640], lhsT=ckgF[:, k, bt, :],
                                     rhs=wfq[:, j, 512:640],
                                     start=(k == 0), stop=(k == KCAT - 1))
        for bt in range(2):
            nc.vector.tensor_copy(out=z_sb[:, bt, :, :]
                                  .rearrange("p c n -> p (c n)"), in_=ps2[bt][:])

        # ln1 stats over each chunk -> AllReduce
        zsq = sqpool.tile([128, 2, 5, 128], F32, tag="zsq", name="zsq")
        nc.vector.tensor_mul(out=zsq[:], in0=z_sb[:], in1=z_sb[:])
        st1 = consts.tile([128, 2, 10], F32, tag="st1")
        nc.vector.reduce_sum(out=st1[:, :, 0:5], in_=z_sb[:], axis=AX.X)
        nc.vector.reduce_sum(out=st1[:, :, 5:10], in_=zsq[:], axis=AX.X)
        nc.sync.dma_start(out=ar1_in.ap().rearrange("(o p) s -> p o s", p=128),
                          in_=st1[:])
        nc.gpsimd.collective_compute(
            "AllReduce", ALU.add, replica_groups=GRP,
            ins=[ar1_in.ap()[:]], outs=[ar1_out.ap()[:]])
        st1r = consts.tile([128, 2, 10], F32, tag="st1r")
        nc.sync.dma_start(out=st1r[:], in_=ar1_out.ap()
                          .rearrange("(o p) s -> p o s", p=128))
        for bt in range(2):
            for ch in range(5):
                mean = bnpool.tile([128, 1], F32, tag="m128", name="mean")
                nc.scalar.activation(out=mean[:], in_=st1r[:, bt, ch:ch + 1],
                                     func=AF.Identity, scale=1.0 / 1024)
                msq = bnpool.tile([128, 1], F32, tag="msq128", name="msq")
                nc.vector.tensor_mul(out=msq[:], in0=mean[:], in1=mean[:])
                var = bnpool.tile([128, 1], F32, tag="v128", name="var")
                nc.scalar.activation(out=var[:], in_=st1r[:, bt, 5 + ch:6 + ch],
                                     func=AF.Identity, scale=1.0 / 1024)
                nc.vector.tensor_sub(out=var[:], in0=var[:], in1=msq[:])
                nc.scalar.activation(out=var[:], in_=var[:], func=AF.Sqrt,
                                     bias=eps128[:], scale=1.0)
                nc.vector.reciprocal(out=var[:], in_=var[:])
                nc.vector.tensor_scalar(out=z_sb[:, bt, ch, :],
                                        in0=z_sb[:, bt, ch, :],
                                        scalar1=mean[:], scalar2=var[:],
                                        op0=ALU.subtract, op1=ALU.mult)

        # ---------- cell math on own columns ----------
        zi = z_sb[:, :, 0, :]; zj = z_sb[:, :, 1, :]; zf = z_sb[:, :, 2, :]
        zo = z_sb[:, :, 3, :]; zom = z_sb[:, :, 4, :]
        nc.scalar.activation(out=zf, in_=zf, func=AF.Sigmoid, bias=F_BIAS)
        nc.scalar.activation(out=zi, in_=zi, func=AF.Sigmoid)
        nc.scalar.activation(out=zj, in_=zj, func=AF.Tanh)
        nc.vector.tensor_mul(out=zf, in0=ccol_sb[:], in1=zf)
        nc.vector.tensor_mul(out=zi, in0=zi, in1=zj)
        nc.vector.tensor_add(out=zf, in0=zf, in1=zi)
        # ln2 on new_c (own 128 cols of the H-chunk) -> AllReduce
        ncsq = sqpool.tile([128, 2, 128], F32, tag="ncsq", name="ncsq")
        nc.vector.tensor_mul(out=ncsq[:], in0=zf, in1=zf)
        st2 = consts.tile([128, 2, 2], F32, tag="st2")
        nc.vector.reduce_sum(out=st2[:, :, 0:1], in_=zf, axis=AX.X)
        nc.vector.reduce_sum(out=st2[:, :, 1:2], in_=ncsq[:], axis=AX.X)
        nc.sync.dma_start(out=ar2_in.ap().rearrange("(o p) s -> p o s", p=128),
                          in_=st2[:])
        nc.gpsimd.collective_compute(
            "AllReduce", ALU.add, replica_groups=GRP,
            ins=[ar2_in.ap()[:]], outs=[ar2_out.ap()[:]])
        st2r = consts.tile([128, 2, 2], F32, tag="st2r")
        nc.sync.dma_start(out=st2r[:], in_=ar2_out.ap()
                          .rearrange("(o p) s -> p o s", p=128))
        for bt in range(2):
            mean = bnpool.tile([128, 1], F32, tag="m128", name="mean")
            nc.scalar.activation(out=mean[:], in_=st2r[:, bt, 0:1],
                                 func=AF.Identity, scale=1.0 / 1024)
            msq = bnpool.tile([128, 1], F32, tag="msq128", name="msq")
            nc.vector.tensor_mul(out=msq[:], in0=mean[:], in1=mean[:])
            var = bnpool.tile([128, 1], F32, tag="v128", name="var")
            nc.scalar.activation(out=var[:], in_=st2r[:, bt, 1:2],
                                 func=AF.Identity, scale=1.0 / 1024)
            nc.vector.tensor_sub(out=var[:], in0=var[:], in1=msq[:])
            nc.scalar.activation(out=var[:], in_=var[:], func=AF.Sqrt,
                                 bias=eps128[:], scale=1.0)
            nc.vector.reciprocal(out=var[:], in_=var[:])
            nc.vector.tensor_scalar(out=zf[:, bt, :], in0=zf[:, bt, :],
                                    scalar1=mean[:], scalar2=var[:],
                                    op0=ALU.subtract, op1=ALU.mult)
        # new_h = tanh(new_c) * sigmoid(o)
        nc.scalar.activation(out=zj, in_=zf, func=AF.Tanh)
        nc.scalar.activation(out=zo, in_=zo, func=AF.Sigmoid)
        nc.vector.tensor_mul(out=zj, in0=zj, in1=zo)
        # r = tanh(h_entry[:, own cols]) * sigmoid(om): select own cols by matmul
        sel_ps = tp_ps.tile([128, 256], F32, tag="tp", name="sel_ps")
        for bt in range(2):
            for kh in range(8):
                nc.tensor.matmul(sel_ps[:, bt * 128:(bt + 1) * 128],
                                 lhsT=s_sb[:, kh, :], rhs=heT_f[:, kh, bt, :],
                                 start=(kh == 0), stop=(kh == 7))
        selT = consts.tile([128, 256], F32, tag="selT")
        nc.vector.tensor_copy(out=selT[:], in_=sel_ps[:])
        hec = consts.tile([128, 2, 128], F32, tag="hec")
        for bt in range(2):
            tpc = tp_ps.tile([128, 128], F32, tag="tp", name="tpc")
            nc.tensor.transpose(tpc[:], selT[:, bt * 128:(bt + 1) * 128], ident[:])
            nc.vector.tensor_copy(out=hec[:, bt, :], in_=tpc[:])
        nc.scalar.activation(out=hec[:], in_=hec[:], func=AF.Tanh)
        nc.scalar.activation(out=zom, in_=zom, func=AF.Sigmoid)
        nc.vector.tensor_mul(out=hec[:], in0=hec[:], in1=zom)

        out_sb = consts.tile([128, 2, 2, 128], F32, tag="out_sb")
        nc.vector.tensor_copy(out=out_sb[:, :, 0, :], in_=zj)
        nc.vector.tensor_copy(out=out_sb[:, :, 1, :], in_=hec[:])
        nc.sync.dma_start(
            out=out_d.ap().rearrange("(o p) (h n) -> p o h n", p=128, n=128),
            in_=out_sb[:])

    nc.compile()
    return nc


_NC = {}


def _get_nc(plain_affine):
    if plain_affine not in _NC:
        _NC[plain_affine] = (build_nc_colsplit() if plain_affine
                             else build_nc(plain_affine))
    return _NC[plain_affine]


def _check_plain_affine(inp):
    try:
        return bool(np.all(inp["bias1"] == 0) and np.all(inp["bias"] == 0)
                    and all(np.all(inp[f"ln{i}_g"] == 1) for i in range(1, 5))
                    and all(np.all(inp[f"ln{i}_b"] == 0) for i in range(1, 5)))
    except KeyError:
        return False


def make_in_maps(inputs, plain_affine):
    inp = {k: np.asarray(v) for k, v in inputs.items()}
    x = inp["x"].astype(np.float32)
    c = inp["c"].astype(np.float32)
    hmem = inp["hmem"].astype(np.float32)
    bf = ml_dtypes.bfloat16

    keysT_pad = np.zeros((128, M), np.float32)
    keysT_pad[:KD] = inp["keys"].astype(np.float32).T
    row_base = (np.arange(BC, dtype=np.uint32) * M).reshape(BC, 1)

    shared = dict(
        keysT_pad=keysT_pad, row_base=row_base,
        fc_w=inp["fc_w"].astype(np.float32),
        fc_b=inp["fc_b"].astype(np.float32),
    )
    va = inp["vec_a"].astype(np.float32).reshape(F)
    va_hi = va.astype(bf)
    shared["veca_hi"] = va_hi
    shared["veca_lo"] = (va - va_hi.astype(np.float32)).astype(bf)
    whm = inp["fc_w"].astype(np.float32)[1600:2624, :]
    whm_hi = whm.astype(bf)
    shared["fchm_hi"] = whm_hi
    shared["fchm_lo"] = (whm - whm_hi.astype(np.float32)).astype(bf)

    W1 = inp["W_full1"].astype(np.float32)
    WF = inp["W_full"].astype(np.float32)
    if plain_affine:
        shared["xT_bf_full"] = np.ascontiguousarray(x.T).astype(bf)
        shared["cT_bf_full"] = np.ascontiguousarray(c.T).astype(bf)
    else:
        shared.update(
            vec_a=va,
            w1bf=W1.astype(bf), wfbf=WF.astype(bf),
            bias1v=inp["bias1"].astype(np.float32),
            biasv=inp["bias"].astype(np.float32),
            ln1g=inp["ln1_g"].astype(np.float32), ln1b=inp["ln1_b"].astype(np.float32),
            ln2g=inp["ln2_g"].astype(np.float32), ln2b=inp["ln2_b"].astype(np.float32),
            ln3g=inp["ln3_g"].astype(np.float32), ln3b=inp["ln3_b"].astype(np.float32),
            ln4g=inp["ln4_g"].astype(np.float32), ln4b=inp["ln4_b"].astype(np.float32),
        )

    in_maps = []
    for cid in range(N_CORES):
        b0 = cid * BC
        xs = x[b0:b0 + BC]
        cs = c[b0:b0 + BC]
        hs = hmem[b0:b0 + BC]                              # [BC, M, H]
        m = dict(shared)
        hT = np.ascontiguousarray(hs.transpose(2, 0, 1).reshape(H, BM))
        hT_hi = hT.astype(bf)
        m["hmemT_hi"] = hT_hi
        m["hmemT_lo"] = (hT - hT_hi.astype(np.float32)).astype(bf)
        m["hmem_flat"] = np.ascontiguousarray(hs.reshape(BM, H))
        m["xT"] = np.ascontiguousarray(xs.T)
        m["cT"] = np.ascontiguousarray(cs.T)
        m["u_t"] = inp["u_t"][b0:b0 + BC].astype(np.float32)
        m["prev"] = inp["prev_read_location"][b0:b0 + BC].astype(np.float32)
        m["gumbel_u"] = inp["gumbel_u"][b0:b0 + BC].astype(np.float32)
        if plain_affine:
            own = np.r_[cid * 128:(cid + 1) * 128]
            m["c_cols"] = np.ascontiguousarray(c[:, own])
            m["w1_cols"] = np.ascontiguousarray(
                W1[:, np.r_[own, 1024 + own]]).astype(bf)
            wf_idx = (np.arange(5)[:, None] * 1024 + own[None, :]).reshape(-1)
            m["wf_cols"] = np.ascontiguousarray(WF[:, wf_idx]).astype(bf)
            s_sel = np.zeros((H, 128), np.float32)
            s_sel[own, np.arange(128)] = 1.0
            m["s_sel"] = s_sel
        else:
            m["xT_bf"] = np.ascontiguousarray(xs.T).astype(bf)
            m["cT_bf"] = np.ascontiguousarray(cs.T).astype(bf)
            m["c_nat"] = cs
        in_maps.append(m)
    return in_maps


_LAST_EXEC_NS = None


def kernel(**inputs):
    global _LAST_EXEC_NS
    import os
    plain_affine = _check_plain_affine({k: np.asarray(v) for k, v in inputs.items()})
    nc = _get_nc(plain_affine)
    in_maps = make_in_maps(inputs, plain_affine)
    trace = bool(int(os.environ.get("KERNEL_TRACE", "0")))
    res = run_bass_kernel_spmd(nc, in_maps, list(range(N_CORES)), trace=trace)
    _LAST_EXEC_NS = res.exec_time_ns
    if plain_affine:
        out = np.zeros((B, 2 * H), np.float32)
        for cid in range(N_CORES):
            rc = np.asarray(res.results[cid]["out"])
            out[:, cid * 128:(cid + 1) * 128] = rc[:, 0:128]
            out[:, H + cid * 128:H + (cid + 1) * 128] = rc[:, 128:256]
    else:
        out = np.concatenate([res.results[i]["out"] for i in range(N_CORES)], axis=0)
    return out.astype(np.float32)


# revision 27
# speedup vs baseline: 1.4904x; 1.4904x over previous
"""ARMIN/TARDIS addressed-memory cell on 8 trn2 NeuronCores.

Data-parallel over batch: each core processes 32 of the 256 batch rows.
Weights are replicated. Score path (content addressing) runs as a 3-pass
bf16 hi/lo-split matmul (argmax margins are as small as 4e-4: bf16 flips
reads, fp32r at ~1e-4 is still too coarse, the 3-pass split gives ~4e-6);
the small q/r/score-dot matmuls run in exact fp32. The two big cell
matmuls run in bf16, and h_entry is gathered in fp32 via indirect DMA so
the r-output stays exact. The x/c part of matmul 1 plus its weight
streaming is interleaved into the score phase (it doesn't depend on the
addressed read), which keeps the DMA rings busy while the PE grinds on
the score matmuls.
"""

import numpy as np
import ml_dtypes
from contextlib import ExitStack

import concourse.bass as bass
import concourse.bacc as bacc
import concourse.tile as tile
from concourse import mybir
from concourse.bass_utils import run_bass_kernel_spmd
from concourse.masks import make_identity

F32 = mybir.dt.float32
BF16 = mybir.dt.bfloat16
U32 = mybir.dt.uint32
AF = mybir.ActivationFunctionType
ALU = mybir.AluOpType
AX = mybir.AxisListType

N_CORES = 8
B, X, H, M, KD = 256, 512, 1024, 128, 64
BC = B // N_CORES        # 32 batch rows per core
F = H // 4               # 256
BM = BC * M              # 4096
EPS = 1e-5
F_BIAS = 1.0
CHUNK = 512              # bm columns per score-path tile
NCHUNK = BM // CHUNK     # 8
NB = CHUNK // M          # batch rows per chunk (4)
KCAT = (X + 2 * H) // 128  # 20 contraction tiles for the cell matmuls


def _bcast_rows(handle_ap, lo, hi, rows=BC):
    """AP that reads dram vector[lo:hi] broadcast across `rows` partitions."""
    src = handle_ap[lo:hi]
    return bass.AP(tensor=src.tensor, offset=src.offset,
                   ap=[[0, rows]] + [list(d) for d in src.ap])


def build_nc(plain_affine):
    """plain_affine: skip LN gamma/beta and bias adds (they are 1/0)."""
    nc = bacc.Bacc("TRN2", target_bir_lowering=False, debug=False)
    P = {}

    def dp(name, shape, dtype=F32, out=False):
        P[name] = nc.declare_dram_parameter(name, list(shape), dtype, isOutput=out)
        return P[name]

    dp("hmemT_hi", [H, BM], BF16)   # [h, b*128+m] bf16 high part
    dp("hmemT_lo", [H, BM], BF16)   # residual (hmemT - hi) in bf16
    dp("hmem_flat", [BM, H])        # natural rows for the gather
    dp("xT", [X, BC]); dp("cT", [H, BC])
    dp("xT_bf", [X, BC], BF16); dp("cT_bf", [H, BC], BF16)
    dp("c_nat", [BC, H])
    dp("u_t", [BC, M]); dp("prev", [BC, M]); dp("gumbel_u", [BC, M])
    dp("keysT_pad", [128, M])
    dp("fc_w", [X + 2 * H + KD + M, F])
    dp("fc_b", [F]); dp("vec_a", [F])
    dp("veca_hi", [F], BF16); dp("veca_lo", [F], BF16)
    dp("fchm_hi", [H, F], BF16)     # fc_w rows 1600:2624 hi/lo split
    dp("fchm_lo", [H, F], BF16)
    dp("row_base", [BC, 1], U32)
    dp("w1bf", [X + 2 * H, 2 * H], BF16)
    dp("wfbf", [X + 2 * H, 5 * H], BF16)
    if not plain_affine:
        dp("bias1v", [2 * H]); dp("biasv", [5 * H])
        dp("ln1g", [5 * H]); dp("ln1b", [5 * H])
        dp("ln2g", [H]); dp("ln2b", [H])
        dp("ln3g", [2 * H]); dp("ln3b", [2 * H])
        dp("ln4g", [M]); dp("ln4b", [M])
    out_d = dp("out", [BC, 2 * H], out=True)
    score_d = nc.dram_tensor("score_bounce", [BM], F32)

    with ExitStack() as ctx:
        tc = ctx.enter_context(tile.TileContext(nc))
        consts = ctx.enter_context(tc.tile_pool(name="consts", bufs=1))
        hpool = ctx.enter_context(tc.tile_pool(name="hpool", bufs=2))
        wspool = ctx.enter_context(tc.tile_pool(name="wspool", bufs=6))
        hfpool = ctx.enter_context(tc.tile_pool(name="hfpool", bufs=2))
        bnpool = ctx.enter_context(tc.tile_pool(name="bnpool", bufs=4))
        zpool = ctx.enter_context(tc.tile_pool(name="zpool", bufs=2))
        bcpool = ctx.enter_context(tc.tile_pool(name="bcpool", bufs=3))
        pre_ps = ctx.enter_context(tc.tile_pool(name="pre_ps", bufs=3, space="PSUM"))
        tp_ps = ctx.enter_context(tc.tile_pool(name="tp_ps", bufs=2, space="PSUM"))
        cell_ps = ctx.enter_context(tc.tile_pool(name="cell_ps", bufs=1, space="PSUM"))

        def ln_rows(x_ap, d, out_ap, g_tile=None, b_tile=None):
            nsub = (d + 511) // 512
            sub = d // nsub
            stats = bnpool.tile([BC, nsub, 6], F32, tag="bn_stats", name="bn_stats")
            for i in range(nsub):
                nc.vector.bn_stats(out=stats[:, i, :],
                                   in_=x_ap[:, i * sub:(i + 1) * sub])
            mv = bnpool.tile([BC, 2], F32, tag="bn_mv", name="bn_mv")
            nc.vector.bn_aggr(out=mv[:], in_=stats[:])
            rstd = bnpool.tile([BC, 1], F32, tag="bn_rstd", name="bn_rstd")
            nc.scalar.activation(out=rstd[:], in_=mv[:, 1:2], func=AF.Sqrt,
                                 bias=eps_t[:], scale=1.0)
            nc.vector.reciprocal(out=rstd[:], in_=rstd[:])
            nc.vector.tensor_scalar(out=out_ap, in0=x_ap, scalar1=mv[:, 0:1],
                                    scalar2=rstd[:], op0=ALU.subtract, op1=ALU.mult)
            if g_tile is not None:
                nc.vector.tensor_mul(out=out_ap, in0=out_ap, in1=g_tile)
                nc.vector.tensor_add(out=out_ap, in0=out_ap, in1=b_tile)

        def bc_tile(pname, lo, hi):
            t = bcpool.tile([BC, hi - lo], F32, tag="bc", name="bc")
            nc.gpsimd.dma_start(out=t[:], in_=_bcast_rows(P[pname].ap(), lo, hi))
            return t

        # ---------- resident constants ----------
        ident = consts.tile([128, 128], F32, tag="ident")
        make_identity(nc, ident[:])
        eps_t = consts.tile([BC, 1], F32, tag="eps")
        nc.vector.memset(eps_t[:], EPS)
        e20_t = consts.tile([BC, 1], F32, tag="e20")
        nc.vector.memset(e20_t[:], 1e-20)

        fchm_hi = consts.tile([128, 8, F], BF16, tag="fchm_hi")
        nc.sync.dma_start(out=fchm_hi[:], in_=P["fchm_hi"].ap()[:, :]
                          .rearrange("(j p) n -> p j n", p=128))
        fchm_lo = consts.tile([128, 8, F], BF16, tag="fchm_lo")
        nc.sync.dma_start(out=fchm_lo[:], in_=P["fchm_lo"].ap()[:, :]
                          .rearrange("(j p) n -> p j n", p=128))
        fcxc = consts.tile([128, 12, F], F32, tag="fcxc")
        nc.sync.dma_start(out=fcxc[:], in_=P["fc_w"].ap()[0:1536, :]
                          .rearrange("(j p) n -> p j n", p=128))
        fckp = consts.tile([128, F], F32, tag="fckp")
        nc.sync.dma_start(out=fckp[:], in_=P["fc_w"].ap()[1536:1664, :])
        fcu = consts.tile([128, F], F32, tag="fcu")
        nc.sync.dma_start(out=fcu[:], in_=P["fc_w"].ap()[2624:2752, :])
        fcb = consts.tile([128, 2], F32, tag="fcb")
        nc.sync.dma_start(out=fcb[:], in_=P["fc_b"].ap().rearrange("(f p) -> p f", p=128))
        veca_hi = consts.tile([128, 2], BF16, tag="veca_hi")
        nc.sync.dma_start(out=veca_hi[:], in_=P["veca_hi"].ap().rearrange("(f p) -> p f", p=128))
        veca_lo = consts.tile([128, 2], BF16, tag="veca_lo")
        nc.sync.dma_start(out=veca_lo[:], in_=P["veca_lo"].ap().rearrange("(f p) -> p f", p=128))
        keysT = consts.tile([128, M], F32, tag="keysT")
        nc.sync.dma_start(out=keysT[:], in_=P["keysT_pad"].ap()[:])

        xT_f = consts.tile([128, 4, BC], F32, tag="xT_f")
        nc.sync.dma_start(out=xT_f[:], in_=P["xT"].ap()[:, :]
                          .rearrange("(j p) n -> p j n", p=128))
        cT_f = consts.tile([128, 8, BC], F32, tag="cT_f")
        nc.sync.dma_start(out=cT_f[:], in_=P["cT"].ap()[:, :]
                          .rearrange("(j p) n -> p j n", p=128))
        ck_bf = consts.tile([128, KCAT, BC], BF16, tag="ck_bf")    # ungated
        ckg_bf = consts.tile([128, KCAT, BC], BF16, tag="ckg_bf")  # gated
        c_nat = consts.tile([BC, H], F32, tag="c_nat")
        u_sb = consts.tile([BC, M], F32, tag="u_sb")
        nc.sync.dma_start(out=u_sb[:], in_=P["u_t"].ap()[:])
        prev_sb = consts.tile([BC, M], F32, tag="prev_sb")
        gum_sb = consts.tile([BC, M], F32, tag="gum_sb")
        rowb = consts.tile([BC, 1], U32, tag="rowb")

        # ---------- u_norm and its transpose ----------
        usq = consts.tile([BC, M], F32, tag="usq")
        nc.scalar.activation(out=usq[:], in_=u_sb[:], func=AF.Square)
        nrm = consts.tile([BC, 1], F32, tag="nrm")
        nc.vector.reduce_sum(out=nrm[:], in_=usq[:], axis=AX.X)
        nc.scalar.activation(out=nrm[:], in_=nrm[:], func=AF.Sqrt)
        nc.vector.tensor_scalar_max(nrm[:], nrm[:], 1e-12)
        nc.vector.reciprocal(out=nrm[:], in_=nrm[:])
        unorm = consts.tile([BC, M], F32, tag="unorm")
        nc.vector.tensor_scalar_mul(unorm[:], u_sb[:], nrm[:])
        tp = tp_ps.tile([128, BC], F32, tag="tp")
        nc.tensor.transpose(tp[:], unorm[:], ident[:BC, :BC])
        unT = consts.tile([128, BC], F32, tag="unT")
        nc.vector.tensor_copy(out=unT[:], in_=tp[:])

        # ---------- q = xc @ W_xc + u_norm @ W_u  (natural [b, f]) ----------
        qps = tp_ps.tile([BC, F], F32, tag="tp", name="qps")
        for k in range(4):
            nc.tensor.matmul(qps[:], lhsT=xT_f[:, k, :], rhs=fcxc[:, k, :],
                             start=(k == 0), stop=False)
        for k in range(8):
            nc.tensor.matmul(qps[:], lhsT=cT_f[:, k, :], rhs=fcxc[:, 4 + k, :],
                             start=False, stop=False)
        nc.tensor.matmul(qps[:], lhsT=unT[:], rhs=fcu[:], start=False, stop=True)
        q_nat = consts.tile([BC, F], F32, tag="q_nat")
        nc.vector.tensor_copy(out=q_nat[:], in_=qps[:])
        qT = consts.tile([128, 2, BC], F32, tag="qT")
        for f in range(2):
            tpq = tp_ps.tile([128, BC], F32, tag="tp", name="tpq")
            nc.tensor.transpose(tpq[:], q_nat[:, f * 128:(f + 1) * 128],
                                ident[:BC, :BC])
            nc.vector.tensor_copy(out=qT[:, f, :], in_=tpq[:])

        # ---------- r_km^T [f, m] = fc_kpad.T @ keysT_pad ----------
        rkT = consts.tile([128, 2, M], F32, tag="rkT")
        for f in range(2):
            rps = tp_ps.tile([128, M], F32, tag="tp", name="rps")
            nc.tensor.matmul(rps[:], lhsT=fckp[:, f * 128:(f + 1) * 128],
                             rhs=keysT[:], start=True, stop=True)
            nc.vector.tensor_copy(out=rkT[:, f, :], in_=rps[:])

        # ---------- score phase: pairs of chunks share each stationary ----------
        for cp in range(NCHUNK // 2):
            hts = []
            for half in range(2):
                ci = cp * 2 + half
                ht_hi = hpool.tile([128, 8, CHUNK], BF16, tag=f"ht_hi{half}",
                                   name="ht_hi")
                nc.sync.dma_start(out=ht_hi[:],
                                  in_=P["hmemT_hi"].ap()[:, ci * CHUNK:(ci + 1) * CHUNK]
                                  .rearrange("(j p) n -> p j n", p=128))
                ht_lo = hpool.tile([128, 8, CHUNK], BF16, tag=f"ht_lo{half}",
                                   name="ht_lo")
                nc.sync.dma_start(out=ht_lo[:],
                                  in_=P["hmemT_lo"].ap()[:, ci * CHUNK:(ci + 1) * CHUNK]
                                  .rearrange("(j p) n -> p j n", p=128))
                hts.append((ht_hi, ht_lo))
            sps2 = [tp_ps.tile([1, CHUNK], F32, tag="tp", name="sps") for _ in range(2)]
            for f in range(2):
                fs = slice(f * 128, (f + 1) * 128)
                ps2c = [pre_ps.tile([128, CHUNK], F32, tag="pre", name="pre")
                        for _ in range(2)]
                for term in range(3):
                    wsl = fchm_hi if term in (0, 2) else fchm_lo
                    for kh in range(8):
                        for half in range(2):
                            rhs = hts[half][1] if term == 2 else hts[half][0]
                            nc.tensor.matmul(ps2c[half][:],
                                             lhsT=wsl[:, kh, fs],
                                             rhs=rhs[:, kh, :],
                                             start=(term == 0 and kh == 0),
                                             stop=(term == 2 and kh == 7))
                for half in range(2):
                    ci = cp * 2 + half
                    ps = ps2c[half]
                    hf = hfpool.tile([128, CHUNK], F32, tag="hf", name="hf")
                    qb = qT[:, f, ci * NB:(ci + 1) * NB, None].to_broadcast(
                        [128, NB, M])
                    nc.vector.tensor_tensor(
                        out=hf[:].rearrange("p (b m) -> p b m", b=NB),
                        in0=ps[:].rearrange("p (b m) -> p b m", b=NB),
                        in1=qb, op=ALU.add)
                    rb = rkT[:, f, None, :].to_broadcast([128, NB, M])
                    nc.vector.tensor_tensor(
                        out=hf[:].rearrange("p (b m) -> p b m", b=NB),
                        in0=hf[:].rearrange("p (b m) -> p b m", b=NB),
                        in1=rb, op=ALU.add)
                    nc.scalar.activation(out=hf[:], in_=hf[:], func=AF.Tanh,
                                         bias=fcb[:, f:f + 1], scale=1.0)
                    hfh = hfpool.tile([128, CHUNK], BF16, tag="hfh", name="hfh")
                    nc.vector.tensor_copy(out=hfh[:], in_=hf[:])
                    hfl = hfpool.tile([128, CHUNK], BF16, tag="hfl", name="hfl")
                    nc.vector.tensor_sub(out=hfl[:], in0=hf[:], in1=hfh[:])
                    nc.tensor.matmul(sps2[half][:], lhsT=veca_hi[:, f:f + 1],
                                     rhs=hfh[:], start=(f == 0), stop=False)
                    nc.tensor.matmul(sps2[half][:], lhsT=veca_lo[:, f:f + 1],
                                     rhs=hfh[:], start=False, stop=False)
                    nc.tensor.matmul(sps2[half][:], lhsT=veca_hi[:, f:f + 1],
                                     rhs=hfl[:], start=False, stop=(f == 1))
            for half in range(2):
                ci = cp * 2 + half
                scs = hfpool.tile([1, CHUNK], F32, tag="scs", name="scs")
                nc.vector.tensor_copy(out=scs[:], in_=sps2[half][:])
                nc.sync.dma_start(
                    out=score_d.ap()[ci * CHUNK:(ci + 1) * CHUNK]
                    .rearrange("(a n) -> a n", a=1),
                    in_=scs[:])

        # deferred cell-phase const loads (keep the score-phase DMA queue lean)
        nc.sync.dma_start(out=ck_bf[:, 0:4, :], in_=P["xT_bf"].ap()[:, :]
                          .rearrange("(j p) n -> p j n", p=128))
        nc.sync.dma_start(out=ckg_bf[:, 0:4, :], in_=P["xT_bf"].ap()[:, :]
                          .rearrange("(j p) n -> p j n", p=128))
        nc.sync.dma_start(out=ck_bf[:, 4:12, :], in_=P["cT_bf"].ap()[:, :]
                          .rearrange("(j p) n -> p j n", p=128))
        nc.sync.dma_start(out=c_nat[:], in_=P["c_nat"].ap()[:])
        nc.sync.dma_start(out=prev_sb[:], in_=P["prev"].ap()[:])
        nc.sync.dma_start(out=gum_sb[:], in_=P["gumbel_u"].ap()[:])
        nc.sync.dma_start(out=rowb[:], in_=P["row_base"].ap()[:])

        # early-emitted weight stream: W1 then Wf prefetch through one pool
        w1_tiles = []
        for kq in range(5):
            for pz in range(2):
                w1t = wspool.tile([128, 4, 1024], BF16, tag="ws", name="w1t")
                nc.sync.dma_start(
                    out=w1t[:],
                    in_=P["w1bf"].ap()[kq * 512:(kq + 1) * 512,
                                       pz * 1024:(pz + 1) * 1024]
                    .rearrange("(j p) n -> p j n", p=128))
                w1_tiles.append(w1t)
        wf_tiles = []
        for pz in range(5):
            for kq in range(5):
                wft = wspool.tile([128, 4, 1024], BF16, tag="ws", name="wft")
                nc.sync.dma_start(
                    out=wft[:],
                    in_=P["wfbf"].ap()[kq * 512:(kq + 1) * 512,
                                       pz * 1024:(pz + 1) * 1024]
                    .rearrange("(j p) n -> p j n", p=128))
                wf_tiles.append(wft)

        # matmul-1 pass 0, x/c contraction tiles: independent of the
        # addressed read, so run them under the argmax/gather tail.
        ps1_0 = cell_ps.tile([BC, 1024], F32, tag="cellps", name="ps1_0")
        for kq in range(3):
            w1t = w1_tiles[kq * 2 + 0]
            for j in range(4):
                k = kq * 4 + j
                for nn in range(2):
                    nc.tensor.matmul(ps1_0[:, nn * 512:(nn + 1) * 512],
                                     lhsT=ck_bf[:, k, :],
                                     rhs=w1t[:, j, nn * 512:(nn + 1) * 512],
                                     start=(k == 0), stop=False)

        score_bm = consts.tile([BC, M], F32, tag="score_bm")
        nc.sync.dma_start(out=score_bm[:],
                          in_=score_d.ap().rearrange("(b m) -> b m", b=BC))

        # score -= prev*100 ; ln4 ; + gumbel ; argmax
        p100 = consts.tile([BC, M], F32, tag="p100")
        nc.vector.tensor_scalar_mul(p100[:], prev_sb[:], 100.0)
        nc.vector.tensor_sub(out=score_bm[:], in0=score_bm[:], in1=p100[:])
        if plain_affine:
            ln_rows(score_bm[:], M, score_bm[:])
        else:
            g4 = bc_tile("ln4g", 0, M)
            b4 = bc_tile("ln4b", 0, M)
            ln_rows(score_bm[:], M, score_bm[:], g4[:], b4[:])
        gt = consts.tile([BC, M], F32, tag="gt")
        nc.scalar.activation(out=gt[:], in_=gum_sb[:], func=AF.Ln, bias=e20_t[:])
        nc.vector.tensor_scalar(out=gt[:], in0=gt[:], scalar1=-1.0, scalar2=1e-20,
                                op0=ALU.mult, op1=ALU.add)
        nc.scalar.activation(out=gt[:], in_=gt[:], func=AF.Ln)
        nc.vector.tensor_sub(out=score_bm[:], in0=score_bm[:], in1=gt[:])
        mx8 = consts.tile([BC, 8], F32, tag="mx8")
        nc.vector.max(out=mx8[:], in_=score_bm[:])
        mi8 = consts.tile([BC, 8], U32, tag="mi8")
        nc.vector.max_index(out=mi8[:], in_max=mx8[:], in_values=score_bm[:])
        flat = consts.tile([BC, 1], U32, tag="flat")
        nc.vector.tensor_tensor(out=flat[:], in0=rowb[:], in1=mi8[:, 0:1], op=ALU.add)

        # gather h_entry rows (fp32 exact)
        h_ent = consts.tile([BC, H], F32, tag="h_ent")
        nc.gpsimd.indirect_dma_start(
            out=h_ent[:], out_offset=None, in_=P["hmem_flat"].ap(),
            in_offset=bass.IndirectOffsetOnAxis(ap=flat[:, :1], axis=0))

        # h_entry^T tiles (fp32 for gating, bf16 for matmul 1)
        hT_f = consts.tile([128, 8, BC], F32, tag="hT_f")
        for kh in range(8):
            tph = tp_ps.tile([128, BC], F32, tag="tp", name="tph")
            nc.tensor.transpose(tph[:], h_ent[:, kh * 128:(kh + 1) * 128],
                                ident[:BC, :BC])
            nc.vector.tensor_copy(out=hT_f[:, kh, :], in_=tph[:])
            nc.vector.tensor_copy(out=ck_bf[:, 12 + kh, :], in_=tph[:])

        # ---------- matmul 1: finish pass 0 (h part), then pass 1 ----------
        g1 = consts.tile([BC, 2 * H], F32, tag="g1")
        for pz in range(2):
            if pz == 0:
                ps1 = ps1_0
                kqs = (3, 4)
            else:
                ps1 = cell_ps.tile([BC, 1024], F32, tag="cellps", name="ps1_1")
                kqs = (0, 1, 2, 3, 4)
            for kq in kqs:
                w1t = w1_tiles[kq * 2 + pz]
                for j in range(4):
                    k = kq * 4 + j
                    for nn in range(2):
                        nc.tensor.matmul(ps1[:, nn * 512:(nn + 1) * 512],
                                         lhsT=ck_bf[:, k, :],
                                         rhs=w1t[:, j, nn * 512:(nn + 1) * 512],
                                         start=(pz == 1 and k == 0),
                                         stop=(k == KCAT - 1))
            z1p = zpool.tile([BC, 1024], F32, tag="z1p", name="z1p")
            if plain_affine:
                nc.vector.tensor_copy(out=z1p[:], in_=ps1[:])
                ln_rows(z1p[:], 1024, z1p[:])
            else:
                b1c = bc_tile("bias1v", pz * 1024, (pz + 1) * 1024)
                nc.vector.tensor_add(out=z1p[:], in0=ps1[:], in1=b1c[:])
                g3 = bc_tile("ln3g", pz * 1024, (pz + 1) * 1024)
                b3 = bc_tile("ln3b", pz * 1024, (pz + 1) * 1024)
                ln_rows(z1p[:], 1024, z1p[:], g3[:], b3[:])
            nc.scalar.activation(out=g1[:, pz * 1024:(pz + 1) * 1024], in_=z1p[:],
                                 func=AF.Sigmoid)

        # gate: ckg[4+t] = (cT | h_entry^T)[t] * g1^T[t]   (bf16 cast on write)
        for t in range(16):
            tpg = tp_ps.tile([128, BC], F32, tag="tp", name="tpg")
            nc.tensor.transpose(tpg[:], g1[:, t * 128:(t + 1) * 128], ident[:BC, :BC])
            src = cT_f[:, t, :] if t < 8 else hT_f[:, t - 8, :]
            nc.vector.tensor_mul(out=ckg_bf[:, 4 + t, :], in0=src, in1=tpg[:])

        # ---------- matmul 2: z = gated @ W_full, ln1 per chunk ----------
        zln = [consts.tile([BC, 1024], F32, tag=f"zln{i}", name=f"zln{i}")
               for i in range(5)]
        for pz in range(5):
            ps2 = cell_ps.tile([BC, 1024], F32, tag="cellps", name="ps2")
            for kq in range(5):
                wft = wf_tiles[pz * 5 + kq]
                for j in range(4):
                    k = kq * 4 + j
                    for nn in range(2):
                        nc.tensor.matmul(ps2[:, nn * 512:(nn + 1) * 512],
                                         lhsT=ckg_bf[:, k, :],
                                         rhs=wft[:, j, nn * 512:(nn + 1) * 512],
                                         start=(k == 0), stop=(k == KCAT - 1))
            if plain_affine:
                nc.vector.tensor_copy(out=zln[pz][:], in_=ps2[:])
                ln_rows(zln[pz][:], 1024, zln[pz][:])
            else:
                bvc = bc_tile("biasv", pz * 1024, (pz + 1) * 1024)
                nc.vector.tensor_add(out=zln[pz][:], in0=ps2[:], in1=bvc[:])
                g1c = bc_tile("ln1g", pz * 1024, (pz + 1) * 1024)
                b1cc = bc_tile("ln1b", pz * 1024, (pz + 1) * 1024)
                ln_rows(zln[pz][:], 1024, zln[pz][:], g1c[:], b1cc[:])

        # ---------- cell math ----------
        zi, zj, zf, zo, zom = zln
        nc.scalar.activation(out=zf[:], in_=zf[:], func=AF.Sigmoid, bias=F_BIAS)
        nc.scalar.activation(out=zi[:], in_=zi[:], func=AF.Sigmoid)
        nc.scalar.activation(out=zj[:], in_=zj[:], func=AF.Tanh)
        nc.vector.tensor_mul(out=zf[:], in0=c_nat[:], in1=zf[:])
        nc.vector.tensor_mul(out=zi[:], in0=zi[:], in1=zj[:])
        nc.vector.tensor_add(out=zf[:], in0=zf[:], in1=zi[:])
        if plain_affine:
            ln_rows(zf[:], H, zf[:])
        else:
            g2c = bc_tile("ln2g", 0, H)
            b2c = bc_tile("ln2b", 0, H)
            ln_rows(zf[:], H, zf[:], g2c[:], b2c[:])
        nc.scalar.activation(out=zj[:], in_=zf[:], func=AF.Tanh)
        nc.scalar.activation(out=zo[:], in_=zo[:], func=AF.Sigmoid)
        nc.vector.tensor_mul(out=zj[:], in0=zj[:], in1=zo[:])
        rh = consts.tile([BC, H], F32, tag="rh")
        nc.scalar.activation(out=rh[:], in_=h_ent[:], func=AF.Tanh)
        nc.scalar.activation(out=zom[:], in_=zom[:], func=AF.Sigmoid)
        nc.vector.tensor_mul(out=rh[:], in0=rh[:], in1=zom[:])

        nc.sync.dma_start(out=out_d.ap()[:, 0:H], in_=zj[:])
        nc.sync.dma_start(out=out_d.ap()[:, H:2 * H], in_=rh[:])

    nc.compile()
    return nc


def build_nc_colsplit():
    """Column-split cell across the 8 cores (plain-affine only).

    Each core computes the full 256-row batch for its own 1/8 of the
    W_full1/W_full columns (within-chunk 128-col blocks, so LN statistics
    AllReduce buffers are identical in shape on every core and the SPMD
    program never depends on the core id). h_entry and the gate are
    AllGathered; the per-core h_entry column slice for the r-output is
    extracted with a host-provided one-hot selection matmul.
    """
    GRP = [list(range(N_CORES))]
    nc = bacc.Bacc("TRN2", target_bir_lowering=False, debug=False,
                   num_devices=N_CORES)
    P = {}

    def dp(name, shape, dtype=F32, out=False):
        P[name] = nc.declare_dram_parameter(name, list(shape), dtype, isOutput=out)
        return P[name]

    dp("hmemT_hi", [H, BM], BF16)
    dp("hmemT_lo", [H, BM], BF16)
    dp("hmem_flat", [BM, H])
    dp("xT", [X, BC]); dp("cT", [H, BC])
    dp("u_t", [BC, M]); dp("prev", [BC, M]); dp("gumbel_u", [BC, M])
    dp("keysT_pad", [128, M])
    dp("fc_w", [X + 2 * H + KD + M, F])
    dp("fc_b", [F])
    dp("veca_hi", [F], BF16); dp("veca_lo", [F], BF16)
    dp("fchm_hi", [H, F], BF16)
    dp("fchm_lo", [H, F], BF16)
    dp("row_base", [BC, 1], U32)
    dp("xT_bf_full", [X, B], BF16)
    dp("cT_bf_full", [H, B], BF16)
    dp("c_cols", [B, 128])
    dp("w1_cols", [X + 2 * H, 256], BF16)
    dp("wf_cols", [X + 2 * H, 640], BF16)
    dp("s_sel", [H, 128])
    out_d = dp("out", [B, 256], out=True)
    score_d = nc.dram_tensor("score_bounce", [BM], F32)
    ag_he_in = nc.dram_tensor("ag_he_in", [BC, H], F32)
    ag_he_out = nc.dram_tensor("ag_he_out", [B, H], F32, addr_space="Shared")
    ag_g1_in = nc.dram_tensor("ag_g1_in", [256, B], F32)
    ag_g1_out = nc.dram_tensor("ag_g1_out", [2048, B], F32, addr_space="Shared")
    ar3_in = nc.dram_tensor("ar3_in", [B, 4], F32)
    ar3_out = nc.dram_tensor("ar3_out", [B, 4], F32, addr_space="Shared")
    ar1_in = nc.dram_tensor("ar1_in", [B, 10], F32)
    ar1_out = nc.dram_tensor("ar1_out", [B, 10], F32, addr_space="Shared")
    ar2_in = nc.dram_tensor("ar2_in", [B, 2], F32)
    ar2_out = nc.dram_tensor("ar2_out", [B, 2], F32, addr_space="Shared")

    with ExitStack() as ctx:
        tc = ctx.enter_context(tile.TileContext(nc))
        consts = ctx.enter_context(tc.tile_pool(name="consts", bufs=1))
        hpool = ctx.enter_context(tc.tile_pool(name="hpool", bufs=2))
        hfpool = ctx.enter_context(tc.tile_pool(name="hfpool", bufs=2))
        bnpool = ctx.enter_context(tc.tile_pool(name="bnpool", bufs=4))
        sqpool = ctx.enter_context(tc.tile_pool(name="sqpool", bufs=1))
        pre_ps = ctx.enter_context(tc.tile_pool(name="pre_ps", bufs=2, space="PSUM"))
        tp_ps = ctx.enter_context(tc.tile_pool(name="tp_ps", bufs=2, space="PSUM"))
        cell_ps = ctx.enter_context(tc.tile_pool(name="cell_ps", bufs=2, space="PSUM"))

        def ln_rows(x_ap, d, out_ap):
            nsub = (d + 511) // 512
            sub = d // nsub
            stats = bnpool.tile([BC, nsub, 6], F32, tag="bn_stats", name="bn_stats")
            for i in range(nsub):
                nc.vector.bn_stats(out=stats[:, i, :],
                                   in_=x_ap[:, i * sub:(i + 1) * sub])
            mv = bnpool.tile([BC, 2], F32, tag="bn_mv", name="bn_mv")
            nc.vector.bn_aggr(out=mv[:], in_=stats[:])
            rstd = bnpool.tile([BC, 1], F32, tag="bn_rstd", name="bn_rstd")
            nc.scalar.activation(out=rstd[:], in_=mv[:, 1:2], func=AF.Sqrt,
                                 bias=eps_t[:], scale=1.0)
            nc.vector.reciprocal(out=rstd[:], in_=rstd[:])
            nc.vector.tensor_scalar(out=out_ap, in0=x_ap, scalar1=mv[:, 0:1],
                                    scalar2=rstd[:], op0=ALU.subtract, op1=ALU.mult)

        # ---------- resident constants (score path) ----------
        ident = consts.tile([128, 128], F32, tag="ident")
        make_identity(nc, ident[:])
        eps_t = consts.tile([BC, 1], F32, tag="eps")
        nc.vector.memset(eps_t[:], EPS)
        eps128 = consts.tile([128, 1], F32, tag="eps128")
        nc.vector.memset(eps128[:], EPS)
        e20_t = consts.tile([BC, 1], F32, tag="e20")
        nc.vector.memset(e20_t[:], 1e-20)

        fchm_hi = consts.tile([128, 8, F], BF16, tag="fchm_hi")
        nc.sync.dma_start(out=fchm_hi[:], in_=P["fchm_hi"].ap()[:, :]
                          .rearrange("(j p) n -> p j n", p=128))
        fchm_lo = consts.tile([128, 8, F], BF16, tag="fchm_lo")
        nc.sync.dma_start(out=fchm_lo[:], in_=P["fchm_lo"].ap()[:, :]
                          .rearrange("(j p) n -> p j n", p=128))
        fcxc = consts.tile([128, 12, F], F32, tag="fcxc")
        nc.sync.dma_start(out=fcxc[:], in_=P["fc_w"].ap()[0:1536, :]
                          .rearrange("(j p) n -> p j n", p=128))
        fckp = consts.tile([128, F], F32, tag="fckp")
        nc.sync.dma_start(out=fckp[:], in_=P["fc_w"].ap()[1536:1664, :])
        fcu = consts.tile([128, F], F32, tag="fcu")
        nc.sync.dma_start(out=fcu[:], in_=P["fc_w"].ap()[2624:2752, :])
        fcb = consts.tile([128, 2], F32, tag="fcb")
        nc.sync.dma_start(out=fcb[:], in_=P["fc_b"].ap().rearrange("(f p) -> p f", p=128))
        veca_hi = consts.tile([128, 2], BF16, tag="veca_hi")
        nc.sync.dma_start(out=veca_hi[:], in_=P["veca_hi"].ap().rearrange("(f p) -> p f", p=128))
        veca_lo = consts.tile([128, 2], BF16, tag="veca_lo")
        nc.sync.dma_start(out=veca_lo[:], in_=P["veca_lo"].ap().rearrange("(f p) -> p f", p=128))
        keysT = consts.tile([128, M], F32, tag="keysT")
        nc.sync.dma_start(out=keysT[:], in_=P["keysT_pad"].ap()[:])
        xT_f = consts.tile([128, 4, BC], F32, tag="xT_f")
        nc.sync.dma_start(out=xT_f[:], in_=P["xT"].ap()[:, :]
                          .rearrange("(j p) n -> p j n", p=128))
        cT_f = consts.tile([128, 8, BC], F32, tag="cT_f")
        nc.sync.dma_start(out=cT_f[:], in_=P["cT"].ap()[:, :]
                          .rearrange("(j p) n -> p j n", p=128))
        u_sb = consts.tile([BC, M], F32, tag="u_sb")
        nc.sync.dma_start(out=u_sb[:], in_=P["u_t"].ap()[:])
        prev_sb = consts.tile([BC, M], F32, tag="prev_sb")
        gum_sb = consts.tile([BC, M], F32, tag="gum_sb")
        rowb = consts.tile([BC, 1], U32, tag="rowb")

        # ---------- u_norm / q / r_km (unchanged score preamble) ----------
        usq = consts.tile([BC, M], F32, tag="usq")
        nc.scalar.activation(out=usq[:], in_=u_sb[:], func=AF.Square)
        nrm = consts.tile([BC, 1], F32, tag="nrm")
        nc.vector.reduce_sum(out=nrm[:], in_=usq[:], axis=AX.X)
        nc.scalar.activation(out=nrm[:], in_=nrm[:], func=AF.Sqrt)
        nc.vector.tensor_scalar_max(nrm[:], nrm[:], 1e-12)
        nc.vector.reciprocal(out=nrm[:], in_=nrm[:])
        unorm = consts.tile([BC, M], F32, tag="unorm")
        nc.vector.tensor_scalar_mul(unorm[:], u_sb[:], nrm[:])
        tp = tp_ps.tile([128, BC], F32, tag="tp")
        nc.tensor.transpose(tp[:], unorm[:], ident[:BC, :BC])
        unT = consts.tile([128, BC], F32, tag="unT")
        nc.vector.tensor_copy(out=unT[:], in_=tp[:])

        qps = tp_ps.tile([BC, F], F32, tag="tp", name="qps")
        for k in range(4):
            nc.tensor.matmul(qps[:], lhsT=xT_f[:, k, :], rhs=fcxc[:, k, :],
                             start=(k == 0), stop=False)
        for k in range(8):
            nc.tensor.matmul(qps[:], lhsT=cT_f[:, k, :], rhs=fcxc[:, 4 + k, :],
                             start=False, stop=False)
        nc.tensor.matmul(qps[:], lhsT=unT[:], rhs=fcu[:], start=False, stop=True)
        q_nat = consts.tile([BC, F], F32, tag="q_nat")
        nc.vector.tensor_copy(out=q_nat[:], in_=qps[:])
        qT = consts.tile([128, 2, BC], F32, tag="qT")
        for f in range(2):
            tpq = tp_ps.tile([128, BC], F32, tag="tp", name="tpq")
            nc.tensor.transpose(tpq[:], q_nat[:, f * 128:(f + 1) * 128],
                                ident[:BC, :BC])
            nc.vector.tensor_copy(out=qT[:, f, :], in_=tpq[:])
        rkT = consts.tile([128, 2, M], F32, tag="rkT")
        for f in range(2):
            rps = tp_ps.tile([128, M], F32, tag="tp", name="rps")
            nc.tensor.matmul(rps[:], lhsT=fckp[:, f * 128:(f + 1) * 128],
                             rhs=keysT[:], start=True, stop=True)
            nc.vector.tensor_copy(out=rkT[:, f, :], in_=rps[:])

        # ---------- score phase (same as data-parallel build) ----------
        for cp in range(NCHUNK // 2):
            hts = []
            for half in range(2):
                ci = cp * 2 + half
                ht_hi = hpool.tile([128, 8, CHUNK], BF16, tag=f"ht_hi{half}",
                                   name="ht_hi")
                nc.sync.dma_start(out=ht_hi[:],
                                  in_=P["hmemT_hi"].ap()[:, ci * CHUNK:(ci + 1) * CHUNK]
                                  .rearrange("(j p) n -> p j n", p=128))
                ht_lo = hpool.tile([128, 8, CHUNK], BF16, tag=f"ht_lo{half}",
                                   name="ht_lo")
                nc.sync.dma_start(out=ht_lo[:],
                                  in_=P["hmemT_lo"].ap()[:, ci * CHUNK:(ci + 1) * CHUNK]
                                  .rearrange("(j p) n -> p j n", p=128))
                hts.append((ht_hi, ht_lo))
            sps2 = [tp_ps.tile([1, CHUNK], F32, tag="tp", name="sps") for _ in range(2)]
            for f in range(2):
                fs = slice(f * 128, (f + 1) * 128)
                ps2c = [pre_ps.tile([128, CHUNK], F32, tag="pre", name="pre")
                        for _ in range(2)]
                for term in range(3):
                    wsl = fchm_hi if term in (0, 2) else fchm_lo
                    for kh in range(8):
                        for half in range(2):
                            rhs = hts[half][1] if term == 2 else hts[half][0]
                            nc.tensor.matmul(ps2c[half][:],
                                             lhsT=wsl[:, kh, fs],
                                             rhs=rhs[:, kh, :],
                                             start=(term == 0 and kh == 0),
                                             stop=(term == 2 and kh == 7))
                for half in range(2):
                    ci = cp * 2 + half
                    ps = ps2c[half]
                    hf = hfpool.tile([128, CHUNK], F32, tag="hf", name="hf")
                    qb = qT[:, f, ci * NB:(ci + 1) * NB, None].to_broadcast(
                        [128, NB, M])
                    nc.vector.tensor_tensor(
                        out=hf[:].rearrange("p (b m) -> p b m", b=NB),
                        in0=ps[:].rearrange("p (b m) -> p b m", b=NB),
                        in1=qb, op=ALU.add)
                    rb = rkT[:, f, None, :].to_broadcast([128, NB, M])
                    nc.vector.tensor_tensor(
                        out=hf[:].rearrange("p (b m) -> p b m", b=NB),
                        in0=hf[:].rearrange("p (b m) -> p b m", b=NB),
                        in1=rb, op=ALU.add)
                    nc.scalar.activation(out=hf[:], in_=hf[:], func=AF.Tanh,
                                         bias=fcb[:, f:f + 1], scale=1.0)
                    hfh = hfpool.tile([128, CHUNK], BF16, tag="hfh", name="hfh")
                    nc.vector.tensor_copy(out=hfh[:], in_=hf[:])
                    hfl = hfpool.tile([128, CHUNK], BF16, tag="hfl", name="hfl")
                    nc.vector.tensor_sub(out=hfl[:], in0=hf[:], in1=hfh[:])
                    nc.tensor.matmul(sps2[half][:], lhsT=veca_hi[:, f:f + 1],
                                     rhs=hfh[:], start=(f == 0), stop=False)
                    nc.tensor.matmul(sps2[half][:], lhsT=veca_lo[:, f:f + 1],
                                     rhs=hfh[:], start=False, stop=False)
                    nc.tensor.matmul(sps2[half][:], lhsT=veca_hi[:, f:f + 1],
                                     rhs=hfl[:], start=False, stop=(f == 1))
            for half in range(2):
                ci = cp * 2 + half
                scs = hfpool.tile([1, CHUNK], F32, tag="scs", name="scs")
                nc.vector.tensor_copy(out=scs[:], in_=sps2[half][:])
                nc.sync.dma_start(
                    out=score_d.ap()[ci * CHUNK:(ci + 1) * CHUNK]
                    .rearrange("(a n) -> a n", a=1),
                    in_=scs[:])

        # ---------- cell-phase constant loads (prefetch during score) ----------
        w1sb = consts.tile([128, 20, 256], BF16, tag="w1sb")
        nc.sync.dma_start(out=w1sb[:], in_=P["w1_cols"].ap()[:, :]
                          .rearrange("(j p) n -> p j n", p=128))
        ckF = consts.tile([128, KCAT, 2, 128], BF16, tag="ckF")
        ckgF = consts.tile([128, KCAT, 2, 128], BF16, tag="ckgF")
        nc.sync.dma_start(out=ckF[:, 0:4, :, :], in_=P["xT_bf_full"].ap()[:, :]
                          .rearrange("(j p) (o n) -> p j o n", p=128, n=128))
        nc.sync.dma_start(out=ckgF[:, 0:4, :, :], in_=P["xT_bf_full"].ap()[:, :]
                          .rearrange("(j p) (o n) -> p j o n", p=128, n=128))
        nc.sync.dma_start(out=ckF[:, 4:12, :, :], in_=P["cT_bf_full"].ap()[:, :]
                          .rearrange("(j p) (o n) -> p j o n", p=128, n=128))
        s_sb = consts.tile([128, 8, 128], F32, tag="s_sb")
        nc.sync.dma_start(out=s_sb[:], in_=P["s_sel"].ap()[:, :]
                          .rearrange("(j p) n -> p j n", p=128))
        ccol_sb = consts.tile([128, 2, 128], F32, tag="ccol_sb")
        nc.sync.dma_start(out=ccol_sb[:], in_=P["c_cols"].ap()[:, :]
                          .rearrange("(o p) n -> p o n", p=128))
        nc.sync.dma_start(out=prev_sb[:], in_=P["prev"].ap()[:])
        nc.sync.dma_start(out=gum_sb[:], in_=P["gumbel_u"].ap()[:])
        nc.sync.dma_start(out=rowb[:], in_=P["row_base"].ap()[:])

        # ---------- score tail: ln4, gumbel, argmax, gather ----------
        score_bm = consts.tile([BC, M], F32, tag="score_bm")
        nc.sync.dma_start(out=score_bm[:],
                          in_=score_d.ap().rearrange("(b m) -> b m", b=BC))
        p100 = consts.tile([BC, M], F32, tag="p100")
        nc.vector.tensor_scalar_mul(p100[:], prev_sb[:], 100.0)
        nc.vector.tensor_sub(out=score_bm[:], in0=score_bm[:], in1=p100[:])
        ln_rows(score_bm[:], M, score_bm[:])
        gt = consts.tile([BC, M], F32, tag="gt")
        nc.scalar.activation(out=gt[:], in_=gum_sb[:], func=AF.Ln, bias=e20_t[:])
        nc.vector.tensor_scalar(out=gt[:], in0=gt[:], scalar1=-1.0, scalar2=1e-20,
                                op0=ALU.mult, op1=ALU.add)
        nc.scalar.activation(out=gt[:], in_=gt[:], func=AF.Ln)
        nc.vector.tensor_sub(out=score_bm[:], in0=score_bm[:], in1=gt[:])
        mx8 = consts.tile([BC, 8], F32, tag="mx8")
        nc.vector.max(out=mx8[:], in_=score_bm[:])
        mi8 = consts.tile([BC, 8], U32, tag="mi8")
        nc.vector.max_index(out=mi8[:], in_max=mx8[:], in_values=score_bm[:])
        flat = consts.tile([BC, 1], U32, tag="flat")
        nc.vector.tensor_tensor(out=flat[:], in0=rowb[:], in1=mi8[:, 0:1], op=ALU.add)
        h_ent = consts.tile([BC, H], F32, tag="h_ent")
        nc.gpsimd.indirect_dma_start(
            out=h_ent[:], out_offset=None, in_=P["hmem_flat"].ap(),
            in_offset=bass.IndirectOffsetOnAxis(ap=flat[:, :1], axis=0))

        # ---------- AllGather h_entry, build full-batch h_entry^T ----------
        nc.sync.dma_start(out=ag_he_in.ap()[:], in_=h_ent[:])
        nc.gpsimd.collective_compute(
            "AllGather", ALU.bypass, replica_groups=GRP,
            ins=[ag_he_in.ap()[:]], outs=[ag_he_out.ap()[:]])
        he_nat = consts.tile([128, 2, H], F32, tag="he_nat")
        nc.sync.dma_start(out=he_nat[:], in_=ag_he_out.ap()
                          .rearrange("(o p) n -> p o n", p=128))
        heT_f = consts.tile([128, 8, 2, 128], F32, tag="heT_f")
        for bt in range(2):
            for kh in range(8):
                tph = tp_ps.tile([128, 128], F32, tag="tp", name="tph")
                nc.tensor.transpose(tph[:], he_nat[:, bt, kh * 128:(kh + 1) * 128],
                                    ident[:])
                nc.vector.tensor_copy(out=heT_f[:, kh, bt, :], in_=tph[:])
                nc.vector.tensor_copy(out=ckF[:, 12 + kh, bt, :], in_=tph[:])

        # ---------- matmul 1 (full batch, own 256 W1 columns) ----------
        z1c = consts.tile([128, 2, 256], F32, tag="z1c")
        for bt in range(2):
            ps1 = cell_ps.tile([128, 256], F32, tag="cellps2", name="ps1")
            for k in range(KCAT):
                nc.tensor.matmul(ps1[:], lhsT=ckF[:, k, bt, :], rhs=w1sb[:, k, :],
                                 start=(k == 0), stop=(k == KCAT - 1))
            nc.vector.tensor_copy(out=z1c[:, bt, :], in_=ps1[:])

        # ln3 stats: own 128 cols per chunk -> AllReduce partial sums
        z1sq = sqpool.tile([128, 2, 256], F32, tag="zsq1", name="z1sq")
        nc.vector.tensor_mul(out=z1sq[:], in0=z1c[:], in1=z1c[:])
        st3 = consts.tile([128, 2, 4], F32, tag="st3")
        nc.vector.reduce_sum(out=st3[:, :, 0:2],
                             in_=z1c[:].rearrange("p o (c n) -> p o c n", c=2),
                             axis=AX.X)
        nc.vector.reduce_sum(out=st3[:, :, 2:4],
                             in_=z1sq[:].rearrange("p o (c n) -> p o c n", c=2),
                             axis=AX.X)
        nc.sync.dma_start(out=ar3_in.ap().rearrange("(o p) s -> p o s", p=128),
                          in_=st3[:])
        nc.gpsimd.collective_compute(
            "AllReduce", ALU.add, replica_groups=GRP,
            ins=[ar3_in.ap()[:]], outs=[ar3_out.ap()[:]])
        st3r = consts.tile([128, 2, 4], F32, tag="st3r")
        nc.sync.dma_start(out=st3r[:], in_=ar3_out.ap()
                          .rearrange("(o p) s -> p o s", p=128))
        g1c = consts.tile([128, 2, 256], F32, tag="g1c")
        for bt in range(2):
            for ch in range(2):
                mean = bnpool.tile([128, 1], F32, tag="m128", name="mean")
                nc.scalar.activation(out=mean[:], in_=st3r[:, bt, ch:ch + 1],
                                     func=AF.Identity, scale=1.0 / 1024)
                msq = bnpool.tile([128, 1], F32, tag="msq128", name="msq")
                nc.vector.tensor_mul(out=msq[:], in0=mean[:], in1=mean[:])
                var = bnpool.tile([128, 1], F32, tag="v128", name="var")
                nc.scalar.activation(out=var[:], in_=st3r[:, bt, 2 + ch:3 + ch],
                                     func=AF.Identity, scale=1.0 / 1024)
                nc.vector.tensor_sub(out=var[:], in0=var[:], in1=msq[:])
                nc.scalar.activation(out=var[:], in_=var[:], func=AF.Sqrt,
                                     bias=eps128[:], scale=1.0)
                nc.vector.reciprocal(out=var[:], in_=var[:])
                sl = z1c[:, bt, ch * 128:(ch + 1) * 128]
                nc.vector.tensor_scalar(out=g1c[:, bt, ch * 128:(ch + 1) * 128],
                                        in0=sl, scalar1=mean[:], scalar2=var[:],
                                        op0=ALU.subtract, op1=ALU.mult)
        nc.scalar.activation(out=g1c[:], in_=g1c[:], func=AF.Sigmoid)

        # ---------- AllGather the gate (transposed) ----------
        g1to = consts.tile([128, 2, 2, 128], F32, tag="g1to")
        for lt in range(2):
            for bt in range(2):
                tpg = tp_ps.tile([128, 128], F32, tag="tp", name="tpg")
                nc.tensor.transpose(tpg[:], g1c[:, bt, lt * 128:(lt + 1) * 128],
                                    ident[:])
                nc.vector.tensor_copy(out=g1to[:, lt, bt, :], in_=tpg[:])
        nc.sync.dma_start(
            out=ag_g1_in.ap().rearrange("(lt p) (bt n) -> p lt bt n", p=128, n=128),
            in_=g1to[:])
        nc.gpsimd.collective_compute(
            "AllGather", ALU.bypass, replica_groups=GRP,
            ins=[ag_g1_in.ap()[:]], outs=[ag_g1_out.ap()[:]])
        g1T_sb = consts.tile([128, 16, 2, 128], BF16, tag="g1T_sb")
        nc.gpsimd.dma_start(out=g1T_sb[:], in_=ag_g1_out.ap()
                            .rearrange("(T p) (o n) -> p T o n", p=128, n=128))

        # gating: t<8 gates c (ckF[4+t]), t>=8 gates h_entry (heT)
        for t in range(16):
            T = 2 * t if t < 8 else 2 * (t - 8) + 1
            for bt in range(2):
                src = ckF[:, 4 + t, bt, :] if t < 8 else heT_f[:, t - 8, bt, :]
                nc.vector.tensor_mul(out=ckgF[:, 4 + t, bt, :], in0=src,
                                     in1=g1T_sb[:, T, bt, :])

        # ---------- matmul 2 (full batch, own 640 Wf columns) ----------
        z_sb = consts.tile([128, 2, 5, 128], F32, tag="z_sb")
        ps2 = [cell_ps.tile([128, 640], F32, tag="cellps2", name=f"ps2_{i}")
               for i in range(2)]
        for kq in range(5):
            wfq = hfpool.tile([128, 4, 640], BF16, tag="wfq", name="wfq")
            nc.sync.dma_start(out=wfq[:],
                              in_=P["wf_cols"].ap()[kq * 512:(kq + 1) * 512, :]
                              .rearrange("(j p) n -> p j n", p=128))
            for j in range(4):
                k = kq * 4 + j
                for bt in range(2):
                    nc.tensor.matmul(ps2[bt][:, 0:512], lhsT=ckgF[:, k, bt, :],
                                     rhs=wfq[:, j, 0:512],
                                     start=(k == 0), stop=(k == KCAT - 1))
                    nc.tensor.matmul(ps2[bt][:, 512:640], lhsT=ckgF[:, k, bt, :],
                                     rhs=wfq[:, j, 512:640],
                                     start=(k == 0), stop=(k == KCAT - 1))
        for bt in range(2):
            nc.vector.tensor_copy(out=z_sb[:, bt, :, :]
                                  .rearrange("p c n -> p (c n)"), in_=ps2[bt][:])

        # ln1 stats over each chunk -> AllReduce
        zsq = sqpool.tile([128, 2, 5, 128], F32, tag="zsq", name="zsq")
        nc.vector.tensor_mul(out=zsq[:], in0=z_sb[:], in1=z_sb[:])
        st1 = consts.tile([128, 2, 10], F32, tag="st1")
        nc.vector.reduce_sum(out=st1[:, :, 0:5], in_=z_sb[:], axis=AX.X)
        nc.vector.reduce_sum(out=st1[:, :, 5:10], in_=zsq[:], axis=AX.X)
        nc.sync.dma_start(out=ar1_in.ap().rearrange("(o p) s -> p o s", p=128),
                          in_=st1[:])
        nc.gpsimd.collective_compute(
            "AllReduce", ALU.add, replica_groups=GRP,
            ins=[ar1_in.ap()[:]], outs=[ar1_out.ap()[:]])
        st1r = consts.tile([128, 2, 10], F32, tag="st1r")
        nc.sync.dma_start(out=st1r[:], in_=ar1_out.ap()
                          .rearrange("(o p) s -> p o s", p=128))
        for bt in range(2):
            for ch in range(5):
                mean = bnpool.tile([128, 1], F32, tag="m128", name="mean")
                nc.scalar.activation(out=mean[:], in_=st1r[:, bt, ch:ch + 1],
                                     func=AF.Identity, scale=1.0 / 1024)
                msq = bnpool.tile([128, 1], F32, tag="msq128", name="msq")
                nc.vector.tensor_mul(out=msq[:], in0=mean[:], in1=mean[:])
                var = bnpool.tile([128, 1], F32, tag="v128", name="var")
                nc.scalar.activation(out=var[:], in_=st1r[:, bt, 5 + ch:6 + ch],
                                     func=AF.Identity, scale=1.0 / 1024)
                nc.vector.tensor_sub(out=var[:], in0=var[:], in1=msq[:])
                nc.scalar.activation(out=var[:], in_=var[:], func=AF.Sqrt,
                                     bias=eps128[:], scale=1.0)
                nc.vector.reciprocal(out=var[:], in_=var[:])
                nc.vector.tensor_scalar(out=z_sb[:, bt, ch, :],
                                        in0=z_sb[:, bt, ch, :],
                                        scalar1=mean[:], scalar2=var[:],
                                        op0=ALU.subtract, op1=ALU.mult)

        # ---------- cell math on own columns ----------
        zi = z_sb[:, :, 0, :]; zj = z_sb[:, :, 1, :]; zf = z_sb[:, :, 2, :]
        zo = z_sb[:, :, 3, :]; zom = z_sb[:, :, 4, :]
        nc.scalar.activation(out=zf, in_=zf, func=AF.Sigmoid, bias=F_BIAS)
        nc.scalar.activation(out=zi, in_=zi, func=AF.Sigmoid)
        nc.scalar.activation(out=zj, in_=zj, func=AF.Tanh)
        nc.vector.tensor_mul(out=zf, in0=ccol_sb[:], in1=zf)
        nc.vector.tensor_mul(out=zi, in0=zi, in1=zj)
        nc.vector.tensor_add(out=zf, in0=zf, in1=zi)
        # ln2 on new_c (own 128 cols of the H-chunk) -> AllReduce
        ncsq = sqpool.tile([128, 2, 128], F32, tag="ncsq", name="ncsq")
        nc.vector.tensor_mul(out=ncsq[:], in0=zf, in1=zf)
        st2 = consts.tile([128, 2, 2], F32, tag="st2")
        nc.vector.reduce_sum(out=st2[:, :, 0:1], in_=zf, axis=AX.X)
        nc.vector.reduce_sum(out=st2[:, :, 1:2], in_=ncsq[:], axis=AX.X)
        nc.sync.dma_start(out=ar2_in.ap().rearrange("(o p) s -> p o s", p=128),
                          in_=st2[:])
        nc.gpsimd.collective_compute(
            "AllReduce", ALU.add, replica_groups=GRP,
            ins=[ar2_in.ap()[:]], outs=[ar2_out.ap()[:]])
        st2r = consts.tile([128, 2, 2], F32, tag="st2r")
        nc.sync.dma_start(out=st2r[:], in_=ar2_out.ap()
                          .rearrange("(o p) s -> p o s", p=128))
        for bt in range(2):
            mean = bnpool.tile([128, 1], F32, tag="m128", name="mean")
            nc.scalar.activation(out=mean[:], in_=st2r[:, bt, 0:1],
                                 func=AF.Identity, scale=1.0 / 1024)
            msq = bnpool.tile([128, 1], F32, tag="msq128", name="msq")
            nc.vector.tensor_mul(out=msq[:], in0=mean[:], in1=mean[:])
            var = bnpool.tile([128, 1], F32, tag="v128", name="var")
            nc.scalar.activation(out=var[:], in_=st2r[:, bt, 1:2],
                                 func=AF.Identity, scale=1.0 / 1024)
            nc.vector.tensor_sub(out=var[:], in0=var[:], in1=msq[:])
            nc.scalar.activation(out=var[:], in_=var[:], func=AF.Sqrt,
                                 bias=eps128[:], scale=1.0)
            nc.vector.reciprocal(out=var[:], in_=var[:])
            nc.vector.tensor_scalar(out=zf[:, bt, :], in0=zf[:, bt, :],
                                    scalar1=mean[:], scalar2=var[:],
                                    op0=ALU.subtract, op1=ALU.mult)
        # new_h = tanh(new_c) * sigmoid(o)
        nc.scalar.activation(out=zj, in_=zf, func=AF.Tanh)
        nc.scalar.activation(out=zo, in_=zo, func=AF.Sigmoid)
        nc.vector.tensor_mul(out=zj, in0=zj, in1=zo)
        # r = tanh(h_entry[:, own cols]) * sigmoid(om): select own cols by matmul
        sel_ps = tp_ps.tile([128, 256], F32, tag="tp", name="sel_ps")
        for bt in range(2):
            for kh in range(8):
                nc.tensor.matmul(sel_ps[:, bt * 128:(bt + 1) * 128],
                                 lhsT=s_sb[:, kh, :], rhs=heT_f[:, kh, bt, :],
                                 start=(kh == 0), stop=(kh == 7))
        selT = consts.tile([128, 256], F32, tag="selT")
        nc.vector.tensor_copy(out=selT[:], in_=sel_ps[:])
        hec = consts.tile([128, 2, 128], F32, tag="hec")
        for bt in range(2):
            tpc = tp_ps.tile([128, 128], F32, tag="tp", name="tpc")
            nc.tensor.transpose(tpc[:], selT[:, bt * 128:(bt + 1) * 128], ident[:])
            nc.vector.tensor_copy(out=hec[:, bt, :], in_=tpc[:])
        nc.scalar.activation(out=hec[:], in_=hec[:], func=AF.Tanh)
        nc.scalar.activation(out=zom, in_=zom, func=AF.Sigmoid)
        nc.vector.tensor_mul(out=hec[:], in0=hec[:], in1=zom)

        out_sb = consts.tile([128, 2, 2, 128], F32, tag="out_sb")
        nc.vector.tensor_copy(out=out_sb[:, :, 0, :], in_=zj)
        nc.vector.tensor_copy(out=out_sb[:, :, 1, :], in_=hec[:])
        nc.sync.dma_start(
            out=out_d.ap().rearrange("(o p) (h n) -> p o h n", p=128, n=128),
            in_=out_sb[:])

    nc.compile()
    return nc


_NC = {}


def _get_nc(plain_affine):
    if plain_affine not in _NC:
        _NC[plain_affine] = build_nc(plain_affine)
    return _NC[plain_affine]


def _check_plain_affine(inp):
    try:
        return bool(np.all(inp["bias1"] == 0) and np.all(inp["bias"] == 0)
                    and all(np.all(inp[f"ln{i}_g"] == 1) for i in range(1, 5))
                    and all(np.all(inp[f"ln{i}_b"] == 0) for i in range(1, 5)))
    except KeyError:
        return False


def make_in_maps(inputs, plain_affine):
    inp = {k: np.asarray(v) for k, v in inputs.items()}
    x = inp["x"].astype(np.float32)
    c = inp["c"].astype(np.float32)
    hmem = inp["hmem"].astype(np.float32)
    bf = ml_dtypes.bfloat16

    keysT_pad = np.zeros((128, M), np.float32)
    keysT_pad[:KD] = inp["keys"].astype(np.float32).T
    row_base = (np.arange(BC, dtype=np.uint32) * M).reshape(BC, 1)

    shared = dict(
        keysT_pad=keysT_pad, row_base=row_base,
        fc_w=inp["fc_w"].astype(np.float32),
        fc_b=inp["fc_b"].astype(np.float32),
    )
    va = inp["vec_a"].astype(np.float32).reshape(F)
    shared["vec_a"] = va
    va_hi = va.astype(bf)
    shared["veca_hi"] = va_hi
    shared["veca_lo"] = (va - va_hi.astype(np.float32)).astype(bf)
    whm = inp["fc_w"].astype(np.float32)[1600:2624, :]
    whm_hi = whm.astype(bf)
    shared["fchm_hi"] = whm_hi
    shared["fchm_lo"] = (whm - whm_hi.astype(np.float32)).astype(bf)

    W1 = inp["W_full1"].astype(np.float32)
    WF = inp["W_full"].astype(np.float32)
    shared["w1bf"] = W1.astype(bf)
    shared["wfbf"] = WF.astype(bf)
    if not plain_affine:
        shared.update(
            vec_a=va,
            bias1v=inp["bias1"].astype(np.float32),
            biasv=inp["bias"].astype(np.float32),
            ln1g=inp["ln1_g"].astype(np.float32), ln1b=inp["ln1_b"].astype(np.float32),
            ln2g=inp["ln2_g"].astype(np.float32), ln2b=inp["ln2_b"].astype(np.float32),
            ln3g=inp["ln3_g"].astype(np.float32), ln3b=inp["ln3_b"].astype(np.float32),
            ln4g=inp["ln4_g"].astype(np.float32), ln4b=inp["ln4_b"].astype(np.float32),
        )

    in_maps = []
    for cid in range(N_CORES):
        b0 = cid * BC
        xs = x[b0:b0 + BC]
        cs = c[b0:b0 + BC]
        hs = hmem[b0:b0 + BC]                              # [BC, M, H]
        m = dict(shared)
        hT = np.ascontiguousarray(hs.transpose(2, 0, 1).reshape(H, BM))
        hT_hi = hT.astype(bf)
        m["hmemT_hi"] = hT_hi
        m["hmemT_lo"] = (hT - hT_hi.astype(np.float32)).astype(bf)
        m["hmem_flat"] = np.ascontiguousarray(hs.reshape(BM, H))
        m["xT"] = np.ascontiguousarray(xs.T)
        m["cT"] = np.ascontiguousarray(cs.T)
        m["u_t"] = inp["u_t"][b0:b0 + BC].astype(np.float32)
        m["prev"] = inp["prev_read_location"][b0:b0 + BC].astype(np.float32)
        m["gumbel_u"] = inp["gumbel_u"][b0:b0 + BC].astype(np.float32)
        m["xT_bf"] = np.ascontiguousarray(xs.T).astype(bf)
        m["cT_bf"] = np.ascontiguousarray(cs.T).astype(bf)
        m["c_nat"] = cs
        in_maps.append(m)
    return in_maps


_LAST_EXEC_NS = None


def kernel(**inputs):
    global _LAST_EXEC_NS
    import os
    plain_affine = _check_plain_affine({k: np.asarray(v) for k, v in inputs.items()})
    nc = _get_nc(plain_affine)
    in_maps = make_in_maps(inputs, plain_affine)
    trace = bool(int(os.environ.get("KERNEL_TRACE", "0")))
    res = run_bass_kernel_spmd(nc, in_maps, list(range(N_CORES)), trace=trace)
    _LAST_EXEC_NS = res.exec_time_ns
    out = np.concatenate([res.results[i]["out"] for i in range(N_CORES)], axis=0)
    return out.astype(np.float32)
